# revision 21
# baseline (speedup 1.0000x reference)
"""Bass/Trainium2 kernel for a binarized NN (BNN) forward pass, data-parallel
over 8 NeuronCores.

Reference semantics (fp32):
    h1 = x @ sign(W1).T;  b1 = sign(h1 - mean(h1, axis=0))        # g=1, b=0
    h2 = b1 @ sign(W2).T; b2 = noisy_sign(h2, u2)                  # BN+sign is
    h3 = b2 @ sign(W3).T; b3 = noisy_sign(h3, u3)                  # identity on +-1
    out = b3 @ sign(W4).T

Math facts exploited:
  * b in {+-1,0} and sign(W) in {+-1} make h2/h3/out exact small integers under
    fp32 PSUM accumulation in any order -> fp8 (e4m3) matmuls on PE are
    bit-exact, enabling DoubleRow perf mode.
  * batchnorm+sign on +-1 inputs is the identity (|mean| < 1), so no batch
    statistics and no cross-core communication are needed for layers 2/3.
  * mean(h1, axis=0) == mean(x, axis=0) @ sign(W1).T -> computed on host in
    float64 (tiny dot), passed in as a per-feature threshold c1.
  * Layer 1 runs as TWO fp16 matmul passes on an exact Dekker split
    x = x_hi + x_lo (12+12 mantissa bits; fp16 operands, incl. subnormals,
    are honored exactly by the PE path - probed). This reproduces the fp32
    h1 to ~1e-7, vs the reference's own ~2e-6 chunked-PSUM rounding noise;
    measured ~0-2 borderline sign differences across the full batch, each
    perturbing one batch row (~0.006 rel err) - far inside the 2e-2 gate.
    fp16 matmuls cost 1 PE cycle/row vs fp32's 4.
  * The stochastic flip (u < 0.5*exp(-h^2/50)) & (|h| <= 50) with h an exact
    even integer depends only on |h| in {0,2,...,50}: precompute on host
    A(u) = smallest even a with p(a) <= u, then flip <=> |h| < A. With
    A' = A-1 (odd) and s = sign-with-0-to-minus(h) = Sign(h - 0.5):
        noisy_sign(h) = Sign(h - s*A')        (h - s*A' is odd, never 0)
    so each chain is ACT Sign -> DVE mult -> DVE subtract -> ACT Sign and
    panels hold +-1 directly (no scaling factors anywhere).

Per-core layout is feature-major ("transposed"): activations live as
[features(partitions), batch(free)], so batch stays on the free dim and no
on-device transposes are needed. Batch 16384 is sharded 2048/core.
"""

from contextlib import ExitStack

import numpy as np

import concourse.bass as bass  # noqa: F401
import concourse.tile as tile
from concourse import bacc, mybir
from concourse.bass_utils import run_bass_kernel_spmd

F32 = mybir.dt.float32
F16 = mybir.dt.float16
BF16 = mybir.dt.bfloat16
FP8 = mybir.dt.float8e4
ALU = mybir.AluOpType
ACTF = mybir.ActivationFunctionType
DR = mybir.MatmulPerfMode.DoubleRow

N_CORES = 8
B = 16384                 # full batch
BC = B // N_CORES         # batch per core
D_IN = 784                # layer-1 input features
D_H = 1024                # hidden features
D_OUT = 10                # output features
K1 = (D_IN + 127) // 128  # 7 k-chunks for layer 1 (6 full + 16 rows)
K1_LAST = D_IN - 128 * (K1 - 1)
KH = D_H // 128           # 8 k-chunks for hidden layers
OC = D_H // 128           # 8 output-feature chunks
HB = BC // 2              # half-batch chain width (1024)

# float32(0.5*exp(-(a*a)/50)) for a = 0,2,...,50, computed with jnp.exp on the
# same jax backend the reference uses (fallback if jax is unavailable here).
_PTABLE_BITS = [
    0x3F000000, 0x3EEC515A, 0x3EB9E4E3, 0x3E79375C, 0x3E0E5ACB, 0x3D8A9501,
    0x3CE5ED93, 0x3C2289CB, 0x3B43D285, 0x3A4909DD, 0x392FE09E, 0x38031DFC,
    0x36A696B8, 0x35345CD8, 0x33A6674D, 0x3202D2C5, 0x302F4A31, 0x2E4824C7,
    0x2C42BB52, 0x2A2173E9, 0x27E4229E, 0x258959AD, 0x230CEE5E, 0x207672F6,
    0x1DB79FE2, 0x1AE92B5E,
]


def _prob_table() -> np.ndarray:
    """p(a) for a = 0,2,...,50, bit-matching the reference's jnp.exp."""
    try:
        import jax.numpy as jnp

        a = np.arange(0, 51, 2, dtype=np.float32)
        p = np.asarray(0.5 * jnp.exp(-(jnp.asarray(a) * a) / (2.0 * 5.0**2)),
                       dtype=np.float32)
        if p.shape == (26,) and np.all(np.diff(p) < 0):
            return p
    except Exception:
        pass
    return np.array(_PTABLE_BITS, dtype=np.uint32).view(np.float32)


def _flip_thresholds(u: np.ndarray, ptable: np.ndarray) -> np.ndarray:
    """A(u): flip <=> |h| < A. A = 52 - 2 * #{a : p(a) <= u}."""
    tab = ptable[::-1].copy()  # ascending: p(50), p(48), ..., p(0)
    idx = np.searchsorted(tab, u, side="right")
    return (52 - 2 * idx).astype(np.int32)


# Batch strips per core (software pipeline): small first strip so the PE
# starts quickly behind the DMA transient, small last strip so the drain
# (noisy-sign chains with no L1 work left to hide them) is short.
WIDTHS = [448, 512, 512, 320, 256]
OFFS = [sum(WIDTHS[:i]) for i in range(len(WIDTHS))]
ST = len(WIDTHS)
SWMAX = max(WIDTHS)

# Packed layer-1 operand: rows 0..783 = x_hi, 784..1567 = x_lo (exact
# Dekker split of x), so the 16-row tail of the 784-dim merges into the
# lo rows: 13 k-chunks instead of 2*7.
D_P = 2 * D_IN            # 1568
KP = (D_P + 127) // 128   # 13
KP_LAST = D_P - 128 * (KP - 1)  # 32


def emit_ns_front(nc, pool, ps, sw, a_ap, nb, use_pool_m, cg_bufs=3):
    """First three ops of v = noisy_sign(h) = Sign(h - s*A'): s, m, w.

    ps holds h (exact even integers). s = Sign(ps - 0.5) in {+-1} (maps
    h==0 to -1 like the reference). a_ap holds A' = A-1 (odd, in [-1,51],
    exact in bf16). Returns the w tile; the final Sign is emitted one
    o-round later (emit_ns_back) so the in-order ACT queue never stalls
    waiting for the DVE middle ops. use_pool_m routes the multiply to the
    otherwise-idle Pool engine (used in the drain periods).
    """
    s = pool.tile([128, SWMAX], BF16, tag="s", bufs=cg_bufs)
    nc.scalar.activation(s[:, :sw], ps[:], ACTF.Sign, bias=nb)
    m = pool.tile([128, SWMAX], BF16, tag="m", bufs=cg_bufs)
    eng = nc.gpsimd if use_pool_m else nc.vector
    eng.tensor_tensor(m[:, :sw], s[:, :sw], a_ap, op=ALU.mult)
    w = pool.tile([128, SWMAX], BF16, tag="w", bufs=cg_bufs)
    nc.vector.tensor_tensor(w[:, :sw], ps[:], m[:, :sw], op=ALU.subtract)
    return w


def emit_ns_back(nc, w, sw, out_ap):
    """Final Sign of the noisy-sign chain: w = h - s*A' is an odd integer,
    never 0; bf16 rounding of large |w| cannot cross 0, so Sign is exact."""
    nc.scalar.activation(out_ap, w[:, :sw], ACTF.Sign)


def build_nc(repeat: int = 1):
    """Build the per-core Bass program (same program on all 8 cores)."""
    nc = bacc.Bacc("TRN2", target_bir_lowering=False, debug=False,
                   num_devices=N_CORES)

    xi = nc.dram_tensor("xi", [D_P, BC], F16, kind="ExternalInput").ap()
    a2 = nc.dram_tensor("a2", [D_H, BC], BF16, kind="ExternalInput").ap()
    a3 = nc.dram_tensor("a3", [D_H, BC], BF16, kind="ExternalInput").ap()
    w1 = nc.dram_tensor("w1", [D_P, D_H], FP8, kind="ExternalInput").ap()
    w2 = nc.dram_tensor("w2", [D_H, D_H], FP8, kind="ExternalInput").ap()
    w3 = nc.dram_tensor("w3", [D_H, D_H], FP8, kind="ExternalInput").ap()
    w4 = nc.dram_tensor("w4", [D_H, 16], FP8, kind="ExternalInput").ap()
    c1 = nc.dram_tensor("c1", [128, OC], F32, kind="ExternalInput").ap()
    out = nc.dram_tensor("out", [D_OUT, BC], F32, kind="ExternalOutput").ap()

    with tile.TileContext(nc) as tc:
        with ExitStack() as ctx:
            consts = ctx.enter_context(tc.tile_pool(name="consts", bufs=1))
            xp = ctx.enter_context(tc.tile_pool(name="xp", bufs=1))
            # Per-strip +-1 panels: written in one period, read the next.
            bp = ctx.enter_context(tc.tile_pool(name="bp", bufs=2))
            l1ps = ctx.enter_context(
                tc.tile_pool(name="l1ps", bufs=4, space="PSUM"))
            l2ps = ctx.enter_context(
                tc.tile_pool(name="l2ps", bufs=2, space="PSUM"))
            l3ps = ctx.enter_context(
                tc.tile_pool(name="l3ps", bufs=1, space="PSUM"))
            l4ps = ctx.enter_context(
                tc.tile_pool(name="l4ps", bufs=1, space="PSUM"))
            tmp2 = ctx.enter_context(tc.tile_pool(name="tmp2", bufs=3))
            tmp3 = ctx.enter_context(tc.tile_pool(name="tmp3", bufs=3))
            l4out = ctx.enter_context(tc.tile_pool(name="l4out", bufs=2))

            w1_t = consts.tile([128, KP, D_H], FP8, tag="w1")
            c1_t = consts.tile([128, OC], F32, tag="c1")
            nb_t = consts.tile([128, 1], F32, tag="nb")
            nc.gpsimd.memset(nb_t[:], -0.5)
            xi_t = xp.tile([128, KP, BC], F16, tag="xi")
            w2_t = consts.tile([128, KH, D_H], FP8, tag="w2")
            w3_t = consts.tile([128, KH, D_H], FP8, tag="w3")
            w4_t = consts.tile([128, KH, 16], FP8, tag="w4")
            a2_t = consts.tile([128, KH, BC], BF16, tag="a2")
            a3_t = consts.tile([128, KH, BC], BF16, tag="a3")

            def x_strip(s):
                """Load xi columns for strip s (2 DMAs: 12 full k-chunks as
                one 3D copy + the 32-row tail chunk)."""
                c0, c1_ = OFFS[s], OFFS[s] + WIDTHS[s]
                nc.sync.dma_start(
                    xi_t[:, :KP - 1, c0:c1_],
                    xi[0:128 * (KP - 1), c0:c1_].rearrange(
                        "(k p) m -> p k m", p=128))
                nc.sync.dma_start(xi_t[:KP_LAST, KP - 1, c0:c1_],
                                  xi[128 * (KP - 1):D_P, c0:c1_])

            def a_strip(a_t, a, s):
                c0, c1_ = OFFS[s], OFFS[s] + WIDTHS[s]
                nc.sync.dma_start(
                    a_t[:, :, c0:c1_],
                    a[:, c0:c1_].rearrange("(k p) m -> p k m", p=128))

            # DMA order = consumption order. Strip 0: interleaved 4-chunk
            # groups of w1/x so the k-major period-0 matmuls chase arrivals;
            # grouping amortizes the ~900ns per-DMA semaphore latency.
            w0 = WIDTHS[0]
            for k0 in range(0, KP - 1, 4):
                k1 = min(k0 + 4, KP - 1)
                nc.sync.dma_start(
                    w1_t[:, k0:k1],
                    w1[k0 * 128:k1 * 128, :].rearrange(
                        "(k p) m -> p k m", p=128))
                nc.sync.dma_start(
                    xi_t[:, k0:k1, 0:w0],
                    xi[k0 * 128:k1 * 128, 0:w0].rearrange(
                        "(k p) m -> p k m", p=128))
            nc.sync.dma_start(w1_t[:KP_LAST, KP - 1],
                              w1[128 * (KP - 1):D_P, :])
            nc.sync.dma_start(xi_t[:KP_LAST, KP - 1, 0:w0],
                              xi[128 * (KP - 1):D_P, 0:w0])
            nc.gpsimd.dma_start(c1_t[:], c1[:, :])
            nc.sync.dma_start(w2_t[:, :, :],
                              w2.rearrange("(k p) m -> p k m", p=128))
            x_strip(1)
            a_strip(a2_t, a2, 0)
            nc.sync.dma_start(w3_t[:, :, :],
                              w3.rearrange("(k p) m -> p k m", p=128))
            nc.sync.dma_start(w4_t[:, :, :],
                              w4.rearrange("(k p) m -> p k m", p=128))
            x_strip(2)
            a_strip(a2_t, a2, 1)
            a_strip(a3_t, a3, 0)
            x_strip(3)
            a_strip(a2_t, a2, 2)
            a_strip(a3_t, a3, 1)
            x_strip(4)
            a_strip(a2_t, a2, 3)
            a_strip(a3_t, a3, 2)
            a_strip(a2_t, a2, 4)
            a_strip(a3_t, a3, 3)
            a_strip(a3_t, a3, 4)

            # Software pipeline over batch strips: period p runs L1(p),
            # L2(p-1), L3(p-2), L4(p-3), interleaved per o-round so PE never
            # waits on the ACT/DVE noisy-sign chains.
            sb1, sb2, sb3 = {}, {}, {}
            for p in range(ST + 3):
                s1, s2, s3, s4 = p, p - 1, p - 2, p - 3
                if 0 <= s1 < ST:
                    sb1[s1] = bp.tile([128, KH, SWMAX], FP8, tag="b1",
                                      name="b1")
                if 0 <= s2 < ST:
                    sb2[s2] = bp.tile([128, KH, SWMAX], FP8, tag="b2",
                                      name="b2")
                if 0 <= s3 < ST:
                    sb3[s3] = bp.tile([128, KH, SWMAX], FP8, tag="b3",
                                      name="b3")
                if 0 <= s4 < ST:
                    ps4 = l4ps.tile([16, SWMAX], F32, tag="mm4")

                if p == 0:
                    # Period 0 is DMA-chased: emit k-major over 4-psum groups
                    # so every psum accumulates each chunk as its DMA lands.
                    w = WIDTHS[0]
                    ssl = slice(OFFS[0], OFFS[0] + w)
                    for og in range(0, OC, 4):
                        pss = [l1ps.tile([128, SWMAX], F32, tag="mm1",
                                         name="ps0") for _ in range(4)]
                        for k in range(KP):
                            kk = KP_LAST if k == KP - 1 else 128
                            for i, o in enumerate(range(og, og + 4)):
                                nc.tensor.matmul(
                                    pss[i][:, :w],
                                    w1_t[:kk, k, o * 128:(o + 1) * 128],
                                    xi_t[:kk, k, ssl],
                                    start=(k == 0),
                                    stop=(k == KP - 1),
                                )
                        for i, o in enumerate(range(og, og + 4)):
                            nc.scalar.activation(sb1[0][:, o, :w],
                                                 pss[i][:, :w], ACTF.Sign,
                                                 bias=c1_t[:, o:o + 1])

                use_pool_m = p >= ST - 2
                pend2 = pend3 = None
                for o in range(OC):
                    osl = slice(o * 128, (o + 1) * 128)
                    if 0 < s1 < ST:
                        w = WIDTHS[s1]
                        ssl = slice(OFFS[s1], OFFS[s1] + w)
                        ps = l1ps.tile([128, SWMAX], F32, tag="mm1")
                        for k in range(KP):
                            kk = KP_LAST if k == KP - 1 else 128
                            nc.tensor.matmul(
                                ps[:, :w],
                                w1_t[:kk, k, osl],
                                xi_t[:kk, k, ssl],
                                start=(k == 0),
                                stop=(k == KP - 1),
                            )
                        # b1 = sign(h1 - mu1); c1 arrives negated so ACT
                        # computes Sign(h + (-mu1)) in one op.
                        nc.scalar.activation(sb1[s1][:, o, :w], ps[:, :w],
                                             ACTF.Sign, bias=c1_t[:, o:o + 1])
                    if 0 <= s2 < ST:
                        w = WIDTHS[s2]
                        ps = l2ps.tile([128, SWMAX], F32, tag="mm2")
                        for kp in range(KH // 2):
                            nc.tensor.matmul(
                                ps[:, :w],
                                w2_t[:, 2 * kp:2 * kp + 2, osl],
                                sb1[s2][:, 2 * kp:2 * kp + 2, :w],
                                start=(kp == 0),
                                stop=(kp == KH // 2 - 1),
                                perf_mode=DR,
                            )
                        wt = emit_ns_front(
                            nc, tmp2, ps[:, :w], w,
                            a2_t[:, o, OFFS[s2]:OFFS[s2] + w], nb_t[:],
                            use_pool_m)
                        if pend2 is not None:
                            emit_ns_back(nc, *pend2)
                        pend2 = (wt, w, sb2[s2][:, o, :w])
                    if 0 <= s3 < ST:
                        w = WIDTHS[s3]
                        ps = l3ps.tile([128, SWMAX], F32, tag="mm3")
                        for kp in range(KH // 2):
                            nc.tensor.matmul(
                                ps[:, :w],
                                w3_t[:, 2 * kp:2 * kp + 2, osl],
                                sb2[s3][:, 2 * kp:2 * kp + 2, :w],
                                start=(kp == 0),
                                stop=(kp == KH // 2 - 1),
                                perf_mode=DR,
                            )
                        wt = emit_ns_front(
                            nc, tmp3, ps[:, :w], w,
                            a3_t[:, o, OFFS[s3]:OFFS[s3] + w], nb_t[:],
                            use_pool_m)
                        if pend3 is not None:
                            emit_ns_back(nc, *pend3)
                        pend3 = (wt, w, sb3[s3][:, o, :w])
                    if 0 <= s4 < ST and o < KH // 2:
                        w = WIDTHS[s4]
                        nc.tensor.matmul(
                            ps4[:, :w],
                            w4_t[:, 2 * o:2 * o + 2, :],
                            sb3[s4][:, 2 * o:2 * o + 2, :w],
                            start=(o == 0),
                            stop=(o == KH // 2 - 1),
                            perf_mode=DR,
                        )
                        if o == KH // 2 - 1:
                            ot = l4out.tile([D_OUT, SWMAX], F32, tag="ot")
                            nc.scalar.activation(ot[:, :w], ps4[:D_OUT, :w],
                                                 ACTF.Copy)
                            nc.sync.dma_start(
                                out[:, OFFS[s4]:OFFS[s4] + w], ot[:, :w])
                if pend2 is not None:
                    emit_ns_back(nc, *pend2)
                if pend3 is not None:
                    emit_ns_back(nc, *pend3)

    nc.compile()
    return nc


_NC_CACHE: dict[int, object] = {}


def _get_nc(repeat: int = 1):
    if repeat not in _NC_CACHE:
        _NC_CACHE[repeat] = build_nc(repeat)
    return _NC_CACHE[repeat]


def make_in_maps(x, u2, u3, W1, W2, W3, W4, **_unused):
    """Host preprocessing -> per-core input dicts."""
    fp8_np = mybir.dt.np(FP8)
    bf16_np = mybir.dt.np(BF16)

    x = np.asarray(x, dtype=np.float32)
    W1b = np.sign(np.asarray(W1, dtype=np.float32))
    # mean(h1, axis=0) = sign(W1) @ mean(x, axis=0), in float64.
    mu1 = (W1b.astype(np.float64) @ x.mean(axis=0, dtype=np.float64)).astype(
        np.float32)
    # negated: the device computes Sign(h + bias) with bias = -mu1
    c1 = np.ascontiguousarray((-mu1).reshape(OC, 128).T)  # [128, OC]

    # Exact Dekker split: x = xh + xl with xh = fp16(x) (12-bit round),
    # xl = fp16(x - xh) (exact except deep-subnormal tails ~2^-25).
    # Packed [xh; xl] rows -> 13 k-chunks; W1 rows are repeated to match.
    x_hi = x.astype(np.float16)
    x_lo = (x - x_hi.astype(np.float32)).astype(np.float16)

    pt = _prob_table()
    a2i = _flip_thresholds(np.asarray(u2), pt) - 1   # A' = A-1, odd
    a3i = _flip_thresholds(np.asarray(u3), pt) - 1

    xit = np.ascontiguousarray(
        np.concatenate([x_hi.T, x_lo.T], axis=0))        # [1568, B] fp16
    a2t = np.ascontiguousarray(a2i.T.astype(bf16_np))    # [1024, B] bf16
    a3t = np.ascontiguousarray(a3i.T.astype(bf16_np))
    w1b8 = W1b.T.astype(fp8_np)
    w1t = np.ascontiguousarray(
        np.concatenate([w1b8, w1b8], axis=0))            # [1568, 1024] fp8
    w2t = np.ascontiguousarray(
        np.sign(np.asarray(W2, np.float32)).T).astype(fp8_np)
    w3t = np.ascontiguousarray(
        np.sign(np.asarray(W3, np.float32)).T).astype(fp8_np)
    w4t = np.zeros((D_H, 16), dtype=fp8_np)
    w4t[:, :D_OUT] = np.ascontiguousarray(
        np.sign(np.asarray(W4, np.float32)).T).astype(fp8_np)

    in_maps = []
    for c in range(N_CORES):
        sl = slice(c * BC, (c + 1) * BC)
        in_maps.append({
            "xi": np.ascontiguousarray(xit[:, sl]),
            "a2": np.ascontiguousarray(a2t[:, sl]),
            "a3": np.ascontiguousarray(a3t[:, sl]),
            "w1": w1t,
            "w2": w2t,
            "w3": w3t,
            "w4": w4t,
            "c1": c1,
        })
    return in_maps


def kernel(x, u2, u3, W1, W2, W3, W4,
           g1=None, b1=None, g2=None, b2=None, g3=None, b3=None):
    for g in (g1, g2, g3):
        assert g is None or np.all(np.asarray(g) > 0), "kernel assumes g > 0"
    for b in (b1, b2, b3):
        assert b is None or np.all(np.asarray(b) == 0), "kernel assumes b == 0"

    nc = _get_nc(repeat=1)
    in_maps = make_in_maps(x, u2, u3, W1, W2, W3, W4)
    res = run_bass_kernel_spmd(nc, in_maps, core_ids=list(range(N_CORES)))

    out = np.empty((B, D_OUT), dtype=np.float32)
    for c in range(N_CORES):
        out[c * BC:(c + 1) * BC, :] = res.results[c]["out"].T
    return out


# revision 56
# speedup vs baseline: 1.3002x; 1.3002x over previous
"""Bass/Trainium2 kernel for a binarized NN (BNN) forward pass, data-parallel
over 8 NeuronCores.

Reference semantics (fp32):
    h1 = x @ sign(W1).T;  b1 = sign(h1 - mean(h1, axis=0))        # g=1, b=0
    h2 = b1 @ sign(W2).T; b2 = noisy_sign(h2, u2)                  # BN+sign is
    h3 = b2 @ sign(W3).T; b3 = noisy_sign(h3, u3)                  # identity on +-1
    out = b3 @ sign(W4).T

Math facts exploited:
  * b in {+-1,0} and sign(W) in {+-1} make h2/h3/out exact small integers under
    fp32 PSUM accumulation in any order -> fp8 (e4m3) matmuls on PE are
    bit-exact, enabling DoubleRow perf mode.
  * batchnorm+sign on +-1 inputs is the identity (|mean| < 1), so no batch
    statistics and no cross-core communication are needed for layers 2/3.
  * mean(h1, axis=0) == mean(x, axis=0) @ sign(W1).T -> computed on host in
    float64 (tiny dot), passed in as a per-feature threshold c1.
  * Layer 1 runs as TWO fp16 matmul passes on an exact Dekker split
    x = x_hi + x_lo (12+12 mantissa bits; fp16 operands, incl. subnormals,
    are honored exactly by the PE path - probed). This reproduces the fp32
    h1 to ~1e-7, vs the reference's own ~2e-6 chunked-PSUM rounding noise;
    measured ~0-2 borderline sign differences across the full batch, each
    perturbing one batch row (~0.006 rel err) - far inside the 2e-2 gate.
    fp16 matmuls cost 1 PE cycle/row vs fp32's 4.
  * The stochastic flip (u < 0.5*exp(-h^2/50)) & (|h| <= 50) with h an exact
    even integer depends only on |h| in {0,2,...,50}: precompute on host
    A(u) = smallest even a with p(a) <= u, then flip <=> |h| < A. With
    A' = A-1 (odd) and s = sign-with-0-to-minus(h) = Sign(h - 0.5):
        noisy_sign(h) = Sign(h - s*A')        (h - s*A' is odd, never 0)
    so each chain is ACT Sign -> DVE mult -> DVE subtract -> ACT Sign and
    panels hold +-1 directly (no scaling factors anywhere).

Per-core layout is feature-major ("transposed"): activations live as
[features(partitions), batch(free)], so batch stays on the free dim and no
on-device transposes are needed. Batch 16384 is sharded 2048/core.
"""

from contextlib import ExitStack

import numpy as np

import concourse.bass as bass  # noqa: F401
import concourse.tile as tile
from concourse import bacc, mybir
from concourse.bass_utils import run_bass_kernel_spmd

F32 = mybir.dt.float32
F16 = mybir.dt.float16
BF16 = mybir.dt.bfloat16
FP8 = mybir.dt.float8e4
ALU = mybir.AluOpType
ACTF = mybir.ActivationFunctionType
DR = mybir.MatmulPerfMode.DoubleRow

N_CORES = 8
B = 16384                 # full batch
BC = B // N_CORES         # batch per core
D_IN = 784                # layer-1 input features
D_H = 1024                # hidden features
D_OUT = 10                # output features
K1 = (D_IN + 127) // 128  # 7 k-chunks for layer 1 (6 full + 16 rows)
K1_LAST = D_IN - 128 * (K1 - 1)
KH = D_H // 128           # 8 k-chunks for hidden layers
OC = D_H // 128           # 8 output-feature chunks
HB = BC // 2              # half-batch chain width (1024)

# float32(0.5*exp(-(a*a)/50)) for a = 0,2,...,50, computed with jnp.exp on the
# same jax backend the reference uses (fallback if jax is unavailable here).
_PTABLE_BITS = [
    0x3F000000, 0x3EEC515A, 0x3EB9E4E3, 0x3E79375C, 0x3E0E5ACB, 0x3D8A9501,
    0x3CE5ED93, 0x3C2289CB, 0x3B43D285, 0x3A4909DD, 0x392FE09E, 0x38031DFC,
    0x36A696B8, 0x35345CD8, 0x33A6674D, 0x3202D2C5, 0x302F4A31, 0x2E4824C7,
    0x2C42BB52, 0x2A2173E9, 0x27E4229E, 0x258959AD, 0x230CEE5E, 0x207672F6,
    0x1DB79FE2, 0x1AE92B5E,
]


def _prob_table() -> np.ndarray:
    """p(a) for a = 0,2,...,50, bit-matching the reference's jnp.exp."""
    try:
        import jax.numpy as jnp

        a = np.arange(0, 51, 2, dtype=np.float32)
        p = np.asarray(0.5 * jnp.exp(-(jnp.asarray(a) * a) / (2.0 * 5.0**2)),
                       dtype=np.float32)
        if p.shape == (26,) and np.all(np.diff(p) < 0):
            return p
    except Exception:
        pass
    return np.array(_PTABLE_BITS, dtype=np.uint32).view(np.float32)


def _flip_thresholds(u: np.ndarray, ptable: np.ndarray) -> np.ndarray:
    """A(u): flip <=> |h| < A. A = 52 - 2 * #{a : p(a) <= u}."""
    tab = ptable[::-1].copy()  # ascending: p(50), p(48), ..., p(0)
    idx = np.searchsorted(tab, u, side="right")
    return (52 - 2 * idx).astype(np.int32)


# Batch strips per core (software pipeline): small first strip so the PE
# starts quickly behind the DMA transient, small last strip so the drain
# (noisy-sign chains with no L1 work left to hide them) is short.
WIDTHS = [512, 512, 448, 320, 256]
OFFS = [sum(WIDTHS[:i]) for i in range(len(WIDTHS))]
ST = len(WIDTHS)
SWMAX = max(WIDTHS)

# Packed layer-1 operand: rows 0..783 = x_hi, 784..1567 = x_lo (exact
# Dekker split of x), so the 16-row tail of the 784-dim merges into the
# lo rows: 13 k-chunks instead of 2*7.
D_P = 2 * D_IN            # 1568
KP = (D_P + 127) // 128   # 13
KP_LAST = D_P - 128 * (KP - 1)  # 32


def emit_ns_front(nc, pool, ps, sw, a_ap, nb, use_hb, cg_bufs=3):
    """First three ops of v = noisy_sign(h) = Sign(h - s*A'): s, m, w.

    ps holds h (exact even integers). s = Sign(ps - 0.5) in {+-1} (maps
    h==0 to -1 like the reference). a_ap holds A' = A-1 (odd, in [-1,51],
    exact in bf16). Returns the w tile; the final Sign is emitted one
    o-round later (emit_ns_back) so the in-order ACT queue never stalls
    waiting for the DVE middle ops.
    """
    s = pool.tile([128, SWMAX], BF16, tag="s", bufs=cg_bufs)
    nc.scalar.activation(s[:, :sw], ps[:], ACTF.Sign, bias=nb)
    m = pool.tile([128, SWMAX], BF16, tag="m", bufs=cg_bufs)
    nc.vector.tensor_tensor(m[:, :sw], s[:, :sw], a_ap, op=ALU.mult)
    w = pool.tile([128, SWMAX], BF16, tag="w", bufs=cg_bufs + 1)
    if use_hb:
        # ACT has slack in the drain: copy h to bf16 so the DVE subtract
        # runs in 2x 16-bit mode. Exact: bf16 holds even ints <= 512
        # exactly, and for |h| > 512 the +-2 rounding cannot flip
        # sign(h - s*A') since |w| >= 461.
        hb = pool.tile([128, SWMAX], BF16, tag="hb", bufs=cg_bufs)
        nc.scalar.activation(hb[:, :sw], ps[:], ACTF.Copy)
        nc.vector.tensor_tensor(w[:, :sw], hb[:, :sw], m[:, :sw],
                                op=ALU.subtract)
    else:
        nc.vector.tensor_tensor(w[:, :sw], ps[:], m[:, :sw],
                                op=ALU.subtract)
    return w


def emit_ns_back(nc, w, sw, out_ap, on_dve=False):
    """Final op of the noisy-sign chain: w = h - s*A' is an odd integer,
    never 0 (bf16 rounding of large |w| cannot cross 0). Either ACT Sign
    or an exact DVE clamp(w, -1, 1) - equal because |w| >= 1."""
    if on_dve:
        nc.vector.tensor_scalar(out_ap, w[:, :sw], 1.0, -1.0,
                                op0=ALU.min, op1=ALU.max)
    else:
        nc.scalar.activation(out_ap, w[:, :sw], ACTF.Sign)


def build_nc(repeat: int = 1):
    """Build the per-core Bass program (same program on all 8 cores)."""
    nc = bacc.Bacc("TRN2", target_bir_lowering=False, debug=False,
                   num_devices=N_CORES)

    xi = nc.dram_tensor("xi", [D_P, BC], F16, kind="ExternalInput").ap()
    a2 = nc.dram_tensor("a2", [D_H, BC], BF16, kind="ExternalInput").ap()
    a3 = nc.dram_tensor("a3", [D_H, BC], BF16, kind="ExternalInput").ap()
    w1 = nc.dram_tensor("w1", [D_P, D_H], FP8, kind="ExternalInput").ap()
    w2 = nc.dram_tensor("w2", [D_H, D_H], FP8, kind="ExternalInput").ap()
    w3 = nc.dram_tensor("w3", [D_H, D_H], FP8, kind="ExternalInput").ap()
    w4 = nc.dram_tensor("w4", [D_H, 16], FP8, kind="ExternalInput").ap()
    c1 = nc.dram_tensor("c1", [128, OC], F32, kind="ExternalInput").ap()
    out = nc.dram_tensor("out", [D_OUT, BC], F32, kind="ExternalOutput").ap()

    with tile.TileContext(nc) as tc:
        with ExitStack() as ctx:
            consts = ctx.enter_context(tc.tile_pool(name="consts", bufs=1))
            xp = ctx.enter_context(tc.tile_pool(name="xp", bufs=1))
            # Per-strip +-1 panels: written in one period, read the next.
            bp = ctx.enter_context(tc.tile_pool(name="bp", bufs=3))
            l1ps = ctx.enter_context(
                tc.tile_pool(name="l1ps", bufs=3, space="PSUM"))
            l2ps = ctx.enter_context(
                tc.tile_pool(name="l2ps", bufs=2, space="PSUM"))
            l3ps = ctx.enter_context(
                tc.tile_pool(name="l3ps", bufs=2, space="PSUM"))
            l4ps = ctx.enter_context(
                tc.tile_pool(name="l4ps", bufs=1, space="PSUM"))
            tmp2 = ctx.enter_context(tc.tile_pool(name="tmp2", bufs=3))
            tmp3 = ctx.enter_context(tc.tile_pool(name="tmp3", bufs=3))
            l4out = ctx.enter_context(tc.tile_pool(name="l4out", bufs=2))

            w1_t = consts.tile([128, KP, D_H], FP8, tag="w1")
            c1_t = consts.tile([128, OC], F32, tag="c1")
            nb_t = consts.tile([128, 1], F32, tag="nb")
            nc.gpsimd.memset(nb_t[:], -0.5)
            xi_t = xp.tile([128, KP, BC], F16, tag="xi")
            w2_t = consts.tile([128, KH, D_H], FP8, tag="w2")
            w3_t = consts.tile([128, KH, D_H], FP8, tag="w3")
            w4_t = consts.tile([128, KH, 16], FP8, tag="w4")
            a2_t = consts.tile([128, KH, BC], BF16, tag="a2")
            a3_t = consts.tile([128, KH, BC], BF16, tag="a3")

            def x_strip(s):
                """Load xi columns for strip s (2 DMAs: 12 full k-chunks as
                one 3D copy + the 32-row tail chunk)."""
                c0, c1_ = OFFS[s], OFFS[s] + WIDTHS[s]
                nc.sync.dma_start(
                    xi_t[:, :KP - 1, c0:c1_],
                    xi[0:128 * (KP - 1), c0:c1_].rearrange(
                        "(k p) m -> p k m", p=128))
                nc.sync.dma_start(xi_t[:KP_LAST, KP - 1, c0:c1_],
                                  xi[128 * (KP - 1):D_P, c0:c1_])

            def a_strip(a_t, a, s):
                c0, c1_ = OFFS[s], OFFS[s] + WIDTHS[s]
                nc.sync.dma_start(
                    a_t[:, :, c0:c1_],
                    a[:, c0:c1_].rearrange("(k p) m -> p k m", p=128))

            # DMA order = consumption order. Strip 0: interleaved 4-chunk
            # groups of w1/x so the k-major period-0 matmuls chase arrivals;
            # grouping amortizes the ~900ns per-DMA semaphore latency.
            w0 = WIDTHS[0]
            nc.sync.dma_start(w1_t[:, 0], w1[0:128, :])
            nc.sync.dma_start(xi_t[:, 0, 0:w0], xi[0:128, 0:w0])
            _groups = [(1, 3), (3, 5), (5, 9), (9, KP - 1)]
            for k0, k1 in _groups:
                nc.sync.dma_start(
                    w1_t[:, k0:k1],
                    w1[k0 * 128:k1 * 128, :].rearrange(
                        "(k p) m -> p k m", p=128))
                nc.sync.dma_start(
                    xi_t[:, k0:k1, 0:w0],
                    xi[k0 * 128:k1 * 128, 0:w0].rearrange(
                        "(k p) m -> p k m", p=128))
            nc.sync.dma_start(w1_t[:KP_LAST, KP - 1],
                              w1[128 * (KP - 1):D_P, :])
            nc.sync.dma_start(xi_t[:KP_LAST, KP - 1, 0:w0],
                              xi[128 * (KP - 1):D_P, 0:w0])
            nc.gpsimd.dma_start(c1_t[:], c1[:, :])
            nc.sync.dma_start(w2_t[:, :, :],
                              w2.rearrange("(k p) m -> p k m", p=128))
            x_strip(1)
            a_strip(a2_t, a2, 0)
            nc.sync.dma_start(w3_t[:, :, :],
                              w3.rearrange("(k p) m -> p k m", p=128))
            nc.sync.dma_start(w4_t[:, :, :],
                              w4.rearrange("(k p) m -> p k m", p=128))
            x_strip(2)
            a_strip(a2_t, a2, 1)
            a_strip(a3_t, a3, 0)
            x_strip(3)
            a_strip(a2_t, a2, 2)
            a_strip(a3_t, a3, 1)
            x_strip(4)
            a_strip(a2_t, a2, 3)
            a_strip(a3_t, a3, 2)
            a_strip(a2_t, a2, 4)
            a_strip(a3_t, a3, 3)
            a_strip(a3_t, a3, 4)

            # Software pipeline over batch strips: period p runs L1(p),
            # L2(p-1), L3(p-2), L4(p-3), interleaved per o-round so PE never
            # waits on the ACT/DVE noisy-sign chains.
            sb1, sb2, sb3 = {}, {}, {}
            for p in range(ST + 3):
                s1, s2, s3, s4 = p, p - 1, p - 2, p - 3
                if 0 <= s1 < ST:
                    sb1[s1] = bp.tile([128, KH, SWMAX], FP8, tag="b1",
                                      name="b1")
                if 0 <= s2 < ST:
                    sb2[s2] = bp.tile([128, KH, SWMAX], FP8, tag="b2",
                                      name="b2")
                if 0 <= s3 < ST:
                    sb3[s3] = bp.tile([128, KH, SWMAX], FP8, tag="b3",
                                      name="b3")
                if 0 <= s4 < ST:
                    ps4 = l4ps.tile([16, SWMAX], F32, tag="mm4")

                if p == 0:
                    # Period 0 is DMA-chased: emit k-major over psum pairs
                    # so both psums accumulate each chunk as its DMA lands.
                    w = WIDTHS[0]
                    ssl = slice(OFFS[0], OFFS[0] + w)
                    for og in range(0, OC, 2):
                        on = 2
                        pss = [l1ps.tile([128, SWMAX], F32, tag="mm1",
                                         name="ps0") for _ in range(on)]
                        for k in range(KP):
                            kk = KP_LAST if k == KP - 1 else 128
                            for i, o in enumerate(range(og, og + on)):
                                nc.tensor.matmul(
                                    pss[i][:, :w],
                                    w1_t[:kk, k, o * 128:(o + 1) * 128],
                                    xi_t[:kk, k, ssl],
                                    start=(k == 0),
                                    stop=(k == KP - 1),
                                )
                        for i, o in enumerate(range(og, og + on)):
                            nc.scalar.activation(sb1[0][:, o, :w],
                                                 pss[i][:, :w], ACTF.Sign,
                                                 bias=c1_t[:, o:o + 1])

                drain_v = (p >= ST - 2) and _os.environ.get(
                    "KDVEV", "1") == "1"
                use_hb = drain_v and _os.environ.get("KHB", "0") == "1"
                pend2 = pend3 = None
                for o in range(OC):
                    osl = slice(o * 128, (o + 1) * 128)
                    if 0 < s1 < ST:
                        w = WIDTHS[s1]
                        ssl = slice(OFFS[s1], OFFS[s1] + w)
                        ps = l1ps.tile([128, SWMAX], F32, tag="mm1")
                        for k in range(KP):
                            kk = KP_LAST if k == KP - 1 else 128
                            nc.tensor.matmul(
                                ps[:, :w],
                                w1_t[:kk, k, osl],
                                xi_t[:kk, k, ssl],
                                start=(k == 0),
                                stop=(k == KP - 1),
                            )
                        # b1 = sign(h1 - mu1); c1 arrives negated so ACT
                        # computes Sign(h + (-mu1)) in one op.
                        nc.scalar.activation(sb1[s1][:, o, :w], ps[:, :w],
                                             ACTF.Sign, bias=c1_t[:, o:o + 1])
                    if 0 <= s2 < ST:
                        w = WIDTHS[s2]
                        ps = l2ps.tile([128, SWMAX], F32, tag="mm2")
                        for kp in range(KH // 2):
                            nc.tensor.matmul(
                                ps[:, :w],
                                w2_t[:, 2 * kp:2 * kp + 2, osl],
                                sb1[s2][:, 2 * kp:2 * kp + 2, :w],
                                start=(kp == 0),
                                stop=(kp == KH // 2 - 1),
                                perf_mode=DR,
                            )
                        wt = emit_ns_front(
                            nc, tmp2, ps[:, :w], w,
                            a2_t[:, o, OFFS[s2]:OFFS[s2] + w], nb_t[:],
                            use_hb)
                        dve_v = drain_v and (o % 2 == 1)
                        if pend2 is not None:
                            emit_ns_back(nc, *pend2)
                        pend2 = (wt, w, sb2[s2][:, o, :w], dve_v)
                    if 0 <= s3 < ST:
                        w = WIDTHS[s3]
                        ps = l3ps.tile([128, SWMAX], F32, tag="mm3")
                        for kp in range(KH // 2):
                            nc.tensor.matmul(
                                ps[:, :w],
                                w3_t[:, 2 * kp:2 * kp + 2, osl],
                                sb2[s3][:, 2 * kp:2 * kp + 2, :w],
                                start=(kp == 0),
                                stop=(kp == KH // 2 - 1),
                                perf_mode=DR,
                            )
                        wt = emit_ns_front(
                            nc, tmp3, ps[:, :w], w,
                            a3_t[:, o, OFFS[s3]:OFFS[s3] + w], nb_t[:],
                            use_hb)
                        dve_v = drain_v and (o % 2 == 0)
                        if pend3 is not None:
                            emit_ns_back(nc, *pend3)
                        pend3 = (wt, w, sb3[s3][:, o, :w], dve_v)
                    if 0 <= s4 < ST and o < KH // 2:
                        w = WIDTHS[s4]
                        nc.tensor.matmul(
                            ps4[:, :w],
                            w4_t[:, 2 * o:2 * o + 2, :],
                            sb3[s4][:, 2 * o:2 * o + 2, :w],
                            start=(o == 0),
                            stop=(o == KH // 2 - 1),
                            perf_mode=DR,
                        )
                        if o == KH // 2 - 1:
                            ot = l4out.tile([D_OUT, SWMAX], F32, tag="ot")
                            nc.scalar.activation(ot[:, :w], ps4[:D_OUT, :w],
                                                 ACTF.Copy)
                            nc.sync.dma_start(
                                out[:, OFFS[s4]:OFFS[s4] + w], ot[:, :w])
                if pend2 is not None:
                    emit_ns_back(nc, *pend2)
                if pend3 is not None:
                    emit_ns_back(nc, *pend3)

    nc.compile()
    return nc


_NC_CACHE: dict[int, object] = {}


def _get_nc(repeat: int = 1):
    if repeat not in _NC_CACHE:
        _NC_CACHE[repeat] = build_nc(repeat)
    return _NC_CACHE[repeat]


def make_in_maps(x, u2, u3, W1, W2, W3, W4, **_unused):
    """Host preprocessing -> per-core input dicts."""
    fp8_np = mybir.dt.np(FP8)
    bf16_np = mybir.dt.np(BF16)

    x = np.asarray(x, dtype=np.float32)
    W1b = np.sign(np.asarray(W1, dtype=np.float32))
    # mean(h1, axis=0) = sign(W1) @ mean(x, axis=0), in float64.
    mu1 = (W1b.astype(np.float64) @ x.mean(axis=0, dtype=np.float64)).astype(
        np.float32)
    # negated: the device computes Sign(h + bias) with bias = -mu1
    c1 = np.ascontiguousarray((-mu1).reshape(OC, 128).T)  # [128, OC]

    # Exact Dekker split: x = xh + xl with xh = fp16(x) (12-bit round),
    # xl = fp16(x - xh) (exact except deep-subnormal tails ~2^-25).
    # Packed [xh; xl] rows -> 13 k-chunks; W1 rows are repeated to match.
    x_hi = x.astype(np.float16)
    x_lo = (x - x_hi.astype(np.float32)).astype(np.float16)

    pt = _prob_table()
    a2i = _flip_thresholds(np.asarray(u2), pt) - 1   # A' = A-1, odd
    a3i = _flip_thresholds(np.asarray(u3), pt) - 1

    xit = np.ascontiguousarray(
        np.concatenate([x_hi.T, x_lo.T], axis=0))        # [1568, B] fp16
    a2t = np.ascontiguousarray(a2i.T.astype(bf16_np))    # [1024, B] bf16
    a3t = np.ascontiguousarray(a3i.T.astype(bf16_np))
    w1b8 = W1b.T.astype(fp8_np)
    w1t = np.ascontiguousarray(
        np.concatenate([w1b8, w1b8], axis=0))            # [1568, 1024] fp8
    w2t = np.ascontiguousarray(
        np.sign(np.asarray(W2, np.float32)).T).astype(fp8_np)
    w3t = np.ascontiguousarray(
        np.sign(np.asarray(W3, np.float32)).T).astype(fp8_np)
    w4t = np.zeros((D_H, 16), dtype=fp8_np)
    w4t[:, :D_OUT] = np.ascontiguousarray(
        np.sign(np.asarray(W4, np.float32)).T).astype(fp8_np)

    in_maps = []
    for c in range(N_CORES):
        sl = slice(c * BC, (c + 1) * BC)
        in_maps.append({
            "xi": np.ascontiguousarray(xit[:, sl]),
            "a2": np.ascontiguousarray(a2t[:, sl]),
            "a3": np.ascontiguousarray(a3t[:, sl]),
            "w1": w1t,
            "w2": w2t,
            "w3": w3t,
            "w4": w4t,
            "c1": c1,
        })
    return in_maps


def kernel(x, u2, u3, W1, W2, W3, W4,
           g1=None, b1=None, g2=None, b2=None, g3=None, b3=None):
    for g in (g1, g2, g3):
        assert g is None or np.all(np.asarray(g) > 0), "kernel assumes g > 0"
    for b in (b1, b2, b3):
        assert b is None or np.all(np.asarray(b) == 0), "kernel assumes b == 0"

    nc = _get_nc(repeat=1)
    in_maps = make_in_maps(x, u2, u3, W1, W2, W3, W4)
    res = run_bass_kernel_spmd(nc, in_maps, core_ids=list(range(N_CORES)))

    out = np.empty((B, D_OUT), dtype=np.float32)
    for c in range(N_CORES):
        out[c * BC:(c + 1) * BC, :] = res.results[c]["out"].T
    return out


# revision 75
# speedup vs baseline: 1.3021x; 1.0015x over previous
"""Bass/Trainium2 kernel for a binarized NN (BNN) forward pass, data-parallel
over 8 NeuronCores.

Reference semantics (fp32):
    h1 = x @ sign(W1).T;  b1 = sign(h1 - mean(h1, axis=0))        # g=1, b=0
    h2 = b1 @ sign(W2).T; b2 = noisy_sign(h2, u2)                  # BN+sign is
    h3 = b2 @ sign(W3).T; b3 = noisy_sign(h3, u3)                  # identity on +-1
    out = b3 @ sign(W4).T

Math facts exploited:
  * b in {+-1,0} and sign(W) in {+-1} make h2/h3/out exact small integers under
    fp32 PSUM accumulation in any order -> fp8 (e4m3) matmuls on PE are
    bit-exact, enabling DoubleRow perf mode.
  * batchnorm+sign on +-1 inputs is the identity (|mean| < 1), so no batch
    statistics and no cross-core communication are needed for layers 2/3.
  * mean(h1, axis=0) == mean(x, axis=0) @ sign(W1).T -> computed on host in
    float64 (tiny dot), passed in as a per-feature threshold c1.
  * Layer 1 runs as TWO fp16 matmul passes on an exact Dekker split
    x = x_hi + x_lo (12+12 mantissa bits; fp16 operands, incl. subnormals,
    are honored exactly by the PE path - probed). This reproduces the fp32
    h1 to ~1e-7, vs the reference's own ~2e-6 chunked-PSUM rounding noise;
    measured ~0-2 borderline sign differences across the full batch, each
    perturbing one batch row (~0.006 rel err) - far inside the 2e-2 gate.
    fp16 matmuls cost 1 PE cycle/row vs fp32's 4.
  * The stochastic flip (u < 0.5*exp(-h^2/50)) & (|h| <= 50) with h an exact
    even integer depends only on |h| in {0,2,...,50}: precompute on host
    A(u) = smallest even a with p(a) <= u, then flip <=> |h| < A. With
    A' = A-1 (odd) and s = sign-with-0-to-minus(h) = Sign(h - 0.5):
        noisy_sign(h) = Sign(h - s*A')        (h - s*A' is odd, never 0)
    so each chain is ACT Sign -> DVE mult -> DVE subtract -> ACT Sign and
    panels hold +-1 directly (no scaling factors anywhere).

Per-core layout is feature-major ("transposed"): activations live as
[features(partitions), batch(free)], so batch stays on the free dim and no
on-device transposes are needed. Batch 16384 is sharded 2048/core.
"""

from contextlib import ExitStack

import numpy as np

import concourse.bass as bass  # noqa: F401
import concourse.tile as tile
from concourse import bacc, mybir
from concourse.bass_utils import run_bass_kernel_spmd

F32 = mybir.dt.float32
F16 = mybir.dt.float16
BF16 = mybir.dt.bfloat16
FP8 = mybir.dt.float8e4
ALU = mybir.AluOpType
ACTF = mybir.ActivationFunctionType
DR = mybir.MatmulPerfMode.DoubleRow

N_CORES = 8
B = 16384                 # full batch
BC = B // N_CORES         # batch per core
D_IN = 784                # layer-1 input features
D_H = 1024                # hidden features
D_OUT = 10                # output features
K1 = (D_IN + 127) // 128  # 7 k-chunks for layer 1 (6 full + 16 rows)
K1_LAST = D_IN - 128 * (K1 - 1)
KH = D_H // 128           # 8 k-chunks for hidden layers
OC = D_H // 128           # 8 output-feature chunks
HB = BC // 2              # half-batch chain width (1024)

# float32(0.5*exp(-(a*a)/50)) for a = 0,2,...,50, computed with jnp.exp on the
# same jax backend the reference uses (fallback if jax is unavailable here).
_PTABLE_BITS = [
    0x3F000000, 0x3EEC515A, 0x3EB9E4E3, 0x3E79375C, 0x3E0E5ACB, 0x3D8A9501,
    0x3CE5ED93, 0x3C2289CB, 0x3B43D285, 0x3A4909DD, 0x392FE09E, 0x38031DFC,
    0x36A696B8, 0x35345CD8, 0x33A6674D, 0x3202D2C5, 0x302F4A31, 0x2E4824C7,
    0x2C42BB52, 0x2A2173E9, 0x27E4229E, 0x258959AD, 0x230CEE5E, 0x207672F6,
    0x1DB79FE2, 0x1AE92B5E,
]


def _prob_table() -> np.ndarray:
    """p(a) for a = 0,2,...,50, bit-matching the reference's jnp.exp."""
    try:
        import jax.numpy as jnp

        a = np.arange(0, 51, 2, dtype=np.float32)
        p = np.asarray(0.5 * jnp.exp(-(jnp.asarray(a) * a) / (2.0 * 5.0**2)),
                       dtype=np.float32)
        if p.shape == (26,) and np.all(np.diff(p) < 0):
            return p
    except Exception:
        pass
    return np.array(_PTABLE_BITS, dtype=np.uint32).view(np.float32)


def _flip_thresholds(u: np.ndarray, ptable: np.ndarray) -> np.ndarray:
    """A(u): flip <=> |h| < A. A = 52 - 2 * #{a : p(a) <= u}."""
    tab = ptable[::-1].copy()  # ascending: p(50), p(48), ..., p(0)
    idx = np.searchsorted(tab, u, side="right")
    return (52 - 2 * idx).astype(np.int32)


# Batch strips per core (software pipeline): small first strip so the PE
# starts quickly behind the DMA transient, small last strip so the drain
# (noisy-sign chains with no L1 work left to hide them) is short.
WIDTHS = [512, 512, 448, 320, 256]
OFFS = [sum(WIDTHS[:i]) for i in range(len(WIDTHS))]
ST = len(WIDTHS)
SWMAX = max(WIDTHS)

# Layer-1 operand: x = three 4-bit base-16 digits of round(x*2^21)
# (12 bits, stored at fp8-e4m3-exact scales d*2^-1 / 2^-5 / 2^-9, rows
# padded 3*784 -> 2560 = 10 DoubleRow chunks at 0.5 cy/row) plus an fp16
# residual (|r| <= 2^-10, exact to ~2^-23, 7 chunks at 1.0) accumulating
# into the SAME psum: 12 chunk-equivalents vs 13 for the Dekker split.
# The residual pass reuses the digit weight tile: its rows 0..783 are
# the plain +-1 signs.
GROWS = 2560              # padded digit rows (3*784 -> 10*256)
NDR = GROWS // 256        # 10 DR chunks
NHC = GROWS // 128        # 20 half-chunks


def emit_ns_front(nc, pool, ps, sw, a_ap, nb, use_hb, cg_bufs=2):
    """First three ops of v = noisy_sign(h) = Sign(h - s*A'): s, m, w.

    ps holds h (exact even integers). s = Sign(ps - 0.5) in {+-1} (maps
    h==0 to -1 like the reference). a_ap holds A' = A-1 (odd, in [-1,51],
    exact in bf16). Returns the w tile; the final Sign is emitted one
    o-round later (emit_ns_back) so the in-order ACT queue never stalls
    waiting for the DVE middle ops.
    """
    s = pool.tile([128, SWMAX], BF16, tag="s", bufs=cg_bufs)
    nc.scalar.activation(s[:, :sw], ps[:], ACTF.Sign, bias=nb)
    m = pool.tile([128, SWMAX], BF16, tag="m", bufs=cg_bufs)
    nc.vector.tensor_tensor(m[:, :sw], s[:, :sw], a_ap, op=ALU.mult)
    w = pool.tile([128, SWMAX], BF16, tag="w", bufs=cg_bufs)
    if use_hb:
        # ACT has slack in the drain: copy h to bf16 so the DVE subtract
        # runs in 2x 16-bit mode. Exact: bf16 holds even ints <= 512
        # exactly, and for |h| > 512 the +-2 rounding cannot flip
        # sign(h - s*A') since |w| >= 461.
        hb = pool.tile([128, SWMAX], BF16, tag="hb", bufs=cg_bufs)
        nc.scalar.activation(hb[:, :sw], ps[:], ACTF.Copy)
        nc.vector.tensor_tensor(w[:, :sw], hb[:, :sw], m[:, :sw],
                                op=ALU.subtract)
    else:
        nc.vector.tensor_tensor(w[:, :sw], ps[:], m[:, :sw],
                                op=ALU.subtract)
    return w


def emit_ns_back(nc, w, sw, out_ap, on_dve=False):
    """Final op of the noisy-sign chain: w = h - s*A' is an odd integer,
    never 0 (bf16 rounding of large |w| cannot cross 0). Either ACT Sign
    or an exact DVE clamp(w, -1, 1) - equal because |w| >= 1."""
    if on_dve:
        nc.vector.tensor_scalar(out_ap, w[:, :sw], 1.0, -1.0,
                                op0=ALU.min, op1=ALU.max)
    else:
        nc.scalar.activation(out_ap, w[:, :sw], ACTF.Sign)


def build_nc(repeat: int = 1):
    """Build the per-core Bass program (same program on all 8 cores)."""
    nc = bacc.Bacc("TRN2", target_bir_lowering=False, debug=False,
                   num_devices=N_CORES)

    xq = nc.dram_tensor("xq", [GROWS, BC], FP8, kind="ExternalInput").ap()
    xr = nc.dram_tensor("xr", [D_IN, BC], F16, kind="ExternalInput").ap()
    a2 = nc.dram_tensor("a2", [D_H, BC], BF16, kind="ExternalInput").ap()
    a3 = nc.dram_tensor("a3", [D_H, BC], BF16, kind="ExternalInput").ap()
    w1 = nc.dram_tensor("w1", [GROWS, D_H], FP8, kind="ExternalInput").ap()
    w2 = nc.dram_tensor("w2", [D_H, D_H], FP8, kind="ExternalInput").ap()
    w3 = nc.dram_tensor("w3", [D_H, D_H], FP8, kind="ExternalInput").ap()
    w4 = nc.dram_tensor("w4", [D_H, 16], FP8, kind="ExternalInput").ap()
    c1 = nc.dram_tensor("c1", [128, OC], F32, kind="ExternalInput").ap()
    out = nc.dram_tensor("out", [D_OUT, BC], F32, kind="ExternalOutput").ap()

    with tile.TileContext(nc) as tc:
        with ExitStack() as ctx:
            consts = ctx.enter_context(tc.tile_pool(name="consts", bufs=1))
            xp = ctx.enter_context(tc.tile_pool(name="xp", bufs=1))
            # Per-strip +-1 panels: written in one period, read the next.
            bp = ctx.enter_context(tc.tile_pool(name="bp", bufs=2))
            l1ps = ctx.enter_context(
                tc.tile_pool(name="l1ps", bufs=3, space="PSUM"))
            l2ps = ctx.enter_context(
                tc.tile_pool(name="l2ps", bufs=2, space="PSUM"))
            l3ps = ctx.enter_context(
                tc.tile_pool(name="l3ps", bufs=2, space="PSUM"))
            l4ps = ctx.enter_context(
                tc.tile_pool(name="l4ps", bufs=1, space="PSUM"))
            tmp2 = ctx.enter_context(tc.tile_pool(name="tmp2", bufs=3))
            tmp3 = ctx.enter_context(tc.tile_pool(name="tmp3", bufs=3))
            l4out = ctx.enter_context(tc.tile_pool(name="l4out", bufs=1))

            w1_t = consts.tile([128, NHC, D_H], FP8, tag="w1")
            c1_t = consts.tile([128, OC], F32, tag="c1")
            nb_t = consts.tile([128, 1], F32, tag="nb")
            nc.gpsimd.memset(nb_t[:], -0.5)
            xq_t = xp.tile([128, NHC, BC], FP8, tag="xq")
            xr_t = xp.tile([128, K1, BC], F16, tag="xr")
            w2_t = consts.tile([128, KH, D_H], FP8, tag="w2")
            w3_t = consts.tile([128, KH, D_H], FP8, tag="w3")
            w4_t = consts.tile([128, KH, 16], FP8, tag="w4")
            a2_t = consts.tile([128, KH, BC], BF16, tag="a2")
            a3_t = consts.tile([128, KH, BC], BF16, tag="a3")

            def x_strip(s):
                """Load digit + residual columns for strip s."""
                c0, c1_ = OFFS[s], OFFS[s] + WIDTHS[s]
                nc.sync.dma_start(
                    xq_t[:, :, c0:c1_],
                    xq[:, c0:c1_].rearrange("(k p) m -> p k m", p=128))
                nc.sync.dma_start(
                    xr_t[:, :K1 - 1, c0:c1_],
                    xr[0:128 * (K1 - 1), c0:c1_].rearrange(
                        "(k p) m -> p k m", p=128))
                nc.sync.dma_start(xr_t[:K1_LAST, K1 - 1, c0:c1_],
                                  xr[128 * (K1 - 1):D_IN, c0:c1_])

            def a_strip(a_t, a, s):
                c0, c1_ = OFFS[s], OFFS[s] + WIDTHS[s]
                nc.sync.dma_start(
                    a_t[:, :, c0:c1_],
                    a[:, c0:c1_].rearrange("(k p) m -> p k m", p=128))

            # DMA order = consumption order. Strip 0: interleaved 4-chunk
            # groups of w1/x so the k-major period-0 matmuls chase arrivals;
            # grouping amortizes the ~900ns per-DMA semaphore latency.
            w0 = WIDTHS[0]
            nc.sync.dma_start(w1_t[:, 0:4],
                              w1[0:512, :].rearrange("(k p) m -> p k m",
                                                     p=128))
            for g in range(5):
                nc.sync.dma_start(
                    xq_t[:, 4 * g:4 * g + 4, 0:w0],
                    xq[512 * g:512 * g + 512, 0:w0].rearrange(
                        "(k p) m -> p k m", p=128))
                if g >= 1:
                    nc.sync.dma_start(
                        w1_t[:, 4 * g:4 * g + 4],
                        w1[512 * g:512 * g + 512, :].rearrange(
                            "(k p) m -> p k m", p=128))
            nc.sync.dma_start(
                xr_t[:, :K1 - 1, 0:w0],
                xr[0:128 * (K1 - 1), 0:w0].rearrange("(k p) m -> p k m",
                                                     p=128))
            nc.sync.dma_start(xr_t[:K1_LAST, K1 - 1, 0:w0],
                              xr[128 * (K1 - 1):D_IN, 0:w0])
            nc.gpsimd.dma_start(c1_t[:], c1[:, :])
            nc.sync.dma_start(w2_t[:, :, :],
                              w2.rearrange("(k p) m -> p k m", p=128))
            x_strip(1)
            a_strip(a2_t, a2, 0)
            nc.sync.dma_start(w3_t[:, :, :],
                              w3.rearrange("(k p) m -> p k m", p=128))
            nc.sync.dma_start(w4_t[:, :, :],
                              w4.rearrange("(k p) m -> p k m", p=128))
            x_strip(2)
            a_strip(a2_t, a2, 1)
            a_strip(a3_t, a3, 0)
            x_strip(3)
            a_strip(a2_t, a2, 2)
            a_strip(a3_t, a3, 1)
            x_strip(4)
            a_strip(a2_t, a2, 3)
            a_strip(a3_t, a3, 2)
            a_strip(a2_t, a2, 4)
            a_strip(a3_t, a3, 3)
            a_strip(a3_t, a3, 4)

            # Software pipeline over batch strips: period p runs L1(p),
            # L2(p-1), L3(p-2), L4(p-3), interleaved per o-round so PE never
            # waits on the ACT/DVE noisy-sign chains.
            sb1, sb2, sb3 = {}, {}, {}
            for p in range(ST + 3):
                s1, s2, s3, s4 = p, p - 1, p - 2, p - 3
                if 0 <= s1 < ST:
                    sb1[s1] = bp.tile([128, KH, SWMAX], FP8, tag="b1",
                                      name="b1")
                if 0 <= s2 < ST:
                    sb2[s2] = bp.tile([128, KH, SWMAX], FP8, tag="b2",
                                      name="b2")
                if 0 <= s3 < ST:
                    sb3[s3] = bp.tile([128, KH, SWMAX], FP8, tag="b3",
                                      name="b3")
                if 0 <= s4 < ST:
                    ps4 = l4ps.tile([16, SWMAX], F32, tag="mm4")

                if p == 0:
                    # Period 0 is DMA-chased: emit k-major over psum pairs
                    # so both psums accumulate each chunk as its DMA lands.
                    w = WIDTHS[0]
                    ssl = slice(OFFS[0], OFFS[0] + w)
                    for og in range(0, OC, 2):
                        on = 2
                        pss = [l1ps.tile([128, SWMAX], F32, tag="mm1",
                                         name="ps0") for _ in range(on)]
                        for k in range(NDR + K1):
                            for i, o in enumerate(range(og, og + on)):
                                osl0 = slice(o * 128, (o + 1) * 128)
                                if k < NDR:
                                    nc.tensor.matmul(
                                        pss[i][:, :w],
                                        w1_t[:, 2 * k:2 * k + 2, osl0],
                                        xq_t[:, 2 * k:2 * k + 2, ssl],
                                        start=(k == 0),
                                        stop=False,
                                        perf_mode=DR,
                                    )
                                else:
                                    c = k - NDR
                                    kk = K1_LAST if c == K1 - 1 else 128
                                    nc.tensor.matmul(
                                        pss[i][:, :w],
                                        w1_t[:kk, c, osl0],
                                        xr_t[:kk, c, ssl],
                                        start=False,
                                        stop=(c == K1 - 1),
                                    )
                        for i, o in enumerate(range(og, og + on)):
                            nc.scalar.activation(sb1[0][:, o, :w],
                                                 pss[i][:, :w], ACTF.Sign,
                                                 bias=c1_t[:, o:o + 1])

                drain_v = (p >= ST - 2) and _os.environ.get(
                    "KDVEV", "1") == "1"
                use_hb = drain_v and _os.environ.get("KHB", "0") == "1"
                pend2 = pend3 = None
                for o in range(OC):
                    osl = slice(o * 128, (o + 1) * 128)
                    if 0 < s1 < ST:
                        w = WIDTHS[s1]
                        ssl = slice(OFFS[s1], OFFS[s1] + w)
                        ps = l1ps.tile([128, SWMAX], F32, tag="mm1")
                        for c in range(NDR):
                            nc.tensor.matmul(
                                ps[:, :w],
                                w1_t[:, 2 * c:2 * c + 2, osl],
                                xq_t[:, 2 * c:2 * c + 2, ssl],
                                start=(c == 0),
                                stop=False,
                                perf_mode=DR,
                            )
                        for c in range(K1):
                            kk = K1_LAST if c == K1 - 1 else 128
                            nc.tensor.matmul(
                                ps[:, :w],
                                w1_t[:kk, c, osl],
                                xr_t[:kk, c, ssl],
                                start=False,
                                stop=(c == K1 - 1),
                            )
                        # b1 = sign(h1 - mu1); c1 arrives negated so ACT
                        # computes Sign(h + (-mu1)) in one op.
                        nc.scalar.activation(sb1[s1][:, o, :w], ps[:, :w],
                                             ACTF.Sign, bias=c1_t[:, o:o + 1])
                    if 0 <= s2 < ST:
                        w = WIDTHS[s2]
                        ps = l2ps.tile([128, SWMAX], F32, tag="mm2")
                        for kp in range(KH // 2):
                            nc.tensor.matmul(
                                ps[:, :w],
                                w2_t[:, 2 * kp:2 * kp + 2, osl],
                                sb1[s2][:, 2 * kp:2 * kp + 2, :w],
                                start=(kp == 0),
                                stop=(kp == KH // 2 - 1),
                                perf_mode=DR,
                            )
                        wt = emit_ns_front(
                            nc, tmp2, ps[:, :w], w,
                            a2_t[:, o, OFFS[s2]:OFFS[s2] + w], nb_t[:],
                            use_hb)
                        dve_v = drain_v and (o % 2 == 1)
                        if pend2 is not None:
                            emit_ns_back(nc, *pend2)
                        pend2 = (wt, w, sb2[s2][:, o, :w], dve_v)
                    if 0 <= s3 < ST:
                        w = WIDTHS[s3]
                        ps = l3ps.tile([128, SWMAX], F32, tag="mm3")
                        for kp in range(KH // 2):
                            nc.tensor.matmul(
                                ps[:, :w],
                                w3_t[:, 2 * kp:2 * kp + 2, osl],
                                sb2[s3][:, 2 * kp:2 * kp + 2, :w],
                                start=(kp == 0),
                                stop=(kp == KH // 2 - 1),
                                perf_mode=DR,
                            )
                        wt = emit_ns_front(
                            nc, tmp3, ps[:, :w], w,
                            a3_t[:, o, OFFS[s3]:OFFS[s3] + w], nb_t[:],
                            use_hb)
                        dve_v = drain_v and (o % 2 == 0)
                        if pend3 is not None:
                            emit_ns_back(nc, *pend3)
                        pend3 = (wt, w, sb3[s3][:, o, :w], dve_v)
                    if 0 <= s4 < ST and o < KH // 2:
                        w = WIDTHS[s4]
                        nc.tensor.matmul(
                            ps4[:, :w],
                            w4_t[:, 2 * o:2 * o + 2, :],
                            sb3[s4][:, 2 * o:2 * o + 2, :w],
                            start=(o == 0),
                            stop=(o == KH // 2 - 1),
                            perf_mode=DR,
                        )
                        if o == KH // 2 - 1:
                            ot = l4out.tile([D_OUT, SWMAX], F32, tag="ot")
                            nc.scalar.activation(ot[:, :w], ps4[:D_OUT, :w],
                                                 ACTF.Copy)
                            nc.sync.dma_start(
                                out[:, OFFS[s4]:OFFS[s4] + w], ot[:, :w])
                if pend2 is not None:
                    emit_ns_back(nc, *pend2)
                if pend3 is not None:
                    emit_ns_back(nc, *pend3)

    nc.compile()
    return nc


_NC_CACHE: dict[int, object] = {}


def _get_nc(repeat: int = 1):
    if repeat not in _NC_CACHE:
        _NC_CACHE[repeat] = build_nc(repeat)
    return _NC_CACHE[repeat]


def make_in_maps(x, u2, u3, W1, W2, W3, W4, **_unused):
    """Host preprocessing -> per-core input dicts."""
    fp8_np = mybir.dt.np(FP8)
    bf16_np = mybir.dt.np(BF16)

    x = np.asarray(x, dtype=np.float32)
    W1b = np.sign(np.asarray(W1, dtype=np.float32))
    # mean(h1, axis=0) = sign(W1) @ mean(x, axis=0), in float64.
    mu1 = (W1b.astype(np.float64) @ x.mean(axis=0, dtype=np.float64)).astype(
        np.float32)
    # negated: the device computes Sign(h + bias) with bias = -mu1
    c1 = np.ascontiguousarray((-mu1).reshape(OC, 128).T)  # [128, OC]

    # Three 4-bit base-16 digits of round(x*2^21) at fp8-exact scales,
    # plus the fp16 residual (|r| <= ~2^-10, exact to ~2^-23).
    q = np.round(x.astype(np.float64) * (1 << 21)).astype(np.int64)
    r = q.T.copy()                                       # [784, B]
    digs = []
    for j in range(3):
        sc = 1 << (4 * (5 - j))
        d = np.floor_divide(2 * r + sc, 2 * sc)          # round-half-up
        r -= d * sc
        digs.append(d)
    assert all(np.abs(d).max() <= 15 for d in digs)
    xqt = np.zeros((GROWS, x.shape[0]), dtype=fp8_np)    # [2560, B]
    for i in range(3):
        v = digs[i].astype(np.float32) * 2.0**(-1 - 4 * i)
        xqt[D_IN * i:D_IN * (i + 1)] = v.astype(fp8_np)
    dig_sum = sum(d.astype(np.float64) * 2.0**(-1 - 4 * i)
                  for i, d in enumerate(digs))
    xrt = (x.T.astype(np.float64) - dig_sum).astype(np.float16)

    pt = _prob_table()
    a2i = _flip_thresholds(np.asarray(u2), pt) - 1   # A' = A-1, odd
    a3i = _flip_thresholds(np.asarray(u3), pt) - 1

    a2t = np.ascontiguousarray(a2i.T.astype(bf16_np))    # [1024, B] bf16
    a3t = np.ascontiguousarray(a3i.T.astype(bf16_np))
    w1b8 = W1b.T.astype(fp8_np)
    w1t = np.zeros((GROWS, D_H), dtype=fp8_np)           # [2560, 1024] fp8
    for i in range(3):
        w1t[D_IN * i:D_IN * (i + 1)] = w1b8
    w2t = np.ascontiguousarray(
        np.sign(np.asarray(W2, np.float32)).T).astype(fp8_np)
    w3t = np.ascontiguousarray(
        np.sign(np.asarray(W3, np.float32)).T).astype(fp8_np)
    w4t = np.zeros((D_H, 16), dtype=fp8_np)
    w4t[:, :D_OUT] = np.ascontiguousarray(
        np.sign(np.asarray(W4, np.float32)).T).astype(fp8_np)

    in_maps = []
    for c in range(N_CORES):
        sl = slice(c * BC, (c + 1) * BC)
        in_maps.append({
            "xq": np.ascontiguousarray(xqt[:, sl]),
            "xr": np.ascontiguousarray(xrt[:, sl]),
            "a2": np.ascontiguousarray(a2t[:, sl]),
            "a3": np.ascontiguousarray(a3t[:, sl]),
            "w1": w1t,
            "w2": w2t,
            "w3": w3t,
            "w4": w4t,
            "c1": c1,
        })
    return in_maps


def kernel(x, u2, u3, W1, W2, W3, W4,
           g1=None, b1=None, g2=None, b2=None, g3=None, b3=None):
    for g in (g1, g2, g3):
        assert g is None or np.all(np.asarray(g) > 0), "kernel assumes g > 0"
    for b in (b1, b2, b3):
        assert b is None or np.all(np.asarray(b) == 0), "kernel assumes b == 0"

    nc = _get_nc(repeat=1)
    in_maps = make_in_maps(x, u2, u3, W1, W2, W3, W4)
    res = run_bass_kernel_spmd(nc, in_maps, core_ids=list(range(N_CORES)))

    out = np.empty((B, D_OUT), dtype=np.float32)
    for c in range(N_CORES):
        out[c * BC:(c + 1) * BC, :] = res.results[c]["out"].T
    return out


# revision 78
# speedup vs baseline: 1.3422x; 1.0308x over previous
"""Bass/Trainium2 kernel for a binarized NN (BNN) forward pass, data-parallel
over 8 NeuronCores.

Reference semantics (fp32):
    h1 = x @ sign(W1).T;  b1 = sign(h1 - mean(h1, axis=0))        # g=1, b=0
    h2 = b1 @ sign(W2).T; b2 = noisy_sign(h2, u2)                  # BN+sign is
    h3 = b2 @ sign(W3).T; b3 = noisy_sign(h3, u3)                  # identity on +-1
    out = b3 @ sign(W4).T

Math facts exploited:
  * b in {+-1,0} and sign(W) in {+-1} make h2/h3/out exact small integers under
    fp32 PSUM accumulation in any order -> fp8 (e4m3) matmuls on PE are
    bit-exact, enabling DoubleRow perf mode.
  * batchnorm+sign on +-1 inputs is the identity (|mean| < 1), so no batch
    statistics and no cross-core communication are needed for layers 2/3.
  * mean(h1, axis=0) == mean(x, axis=0) @ sign(W1).T -> computed on host in
    float64 (tiny dot), passed in as a per-feature threshold c1.
  * Layer 1 runs as TWO fp16 matmul passes on an exact Dekker split
    x = x_hi + x_lo (12+12 mantissa bits; fp16 operands, incl. subnormals,
    are honored exactly by the PE path - probed). This reproduces the fp32
    h1 to ~1e-7, vs the reference's own ~2e-6 chunked-PSUM rounding noise;
    measured ~0-2 borderline sign differences across the full batch, each
    perturbing one batch row (~0.006 rel err) - far inside the 2e-2 gate.
    fp16 matmuls cost 1 PE cycle/row vs fp32's 4.
  * The stochastic flip (u < 0.5*exp(-h^2/50)) & (|h| <= 50) with h an exact
    even integer depends only on |h| in {0,2,...,50}: precompute on host
    A(u) = smallest even a with p(a) <= u, then flip <=> |h| < A. With
    A' = A-1 (odd) and s = sign-with-0-to-minus(h) = Sign(h - 0.5):
        noisy_sign(h) = Sign(h - s*A')        (h - s*A' is odd, never 0)
    so each chain is ACT Sign -> DVE mult -> DVE subtract -> ACT Sign and
    panels hold +-1 directly (no scaling factors anywhere).

Per-core layout is feature-major ("transposed"): activations live as
[features(partitions), batch(free)], so batch stays on the free dim and no
on-device transposes are needed. Batch 16384 is sharded 2048/core.
"""

from contextlib import ExitStack

import numpy as np

import concourse.bass as bass  # noqa: F401
import concourse.tile as tile
from concourse import bacc, mybir
from concourse.bass_utils import run_bass_kernel_spmd

F32 = mybir.dt.float32
F16 = mybir.dt.float16
BF16 = mybir.dt.bfloat16
FP8 = mybir.dt.float8e4
ALU = mybir.AluOpType
ACTF = mybir.ActivationFunctionType
DR = mybir.MatmulPerfMode.DoubleRow

N_CORES = 8
B = 16384                 # full batch
BC = B // N_CORES         # batch per core
D_IN = 784                # layer-1 input features
D_H = 1024                # hidden features
D_OUT = 10                # output features
K1 = (D_IN + 127) // 128  # 7 k-chunks for layer 1 (6 full + 16 rows)
K1_LAST = D_IN - 128 * (K1 - 1)
KH = D_H // 128           # 8 k-chunks for hidden layers
OC = D_H // 128           # 8 output-feature chunks
HB = BC // 2              # half-batch chain width (1024)

# float32(0.5*exp(-(a*a)/50)) for a = 0,2,...,50, computed with jnp.exp on the
# same jax backend the reference uses (fallback if jax is unavailable here).
_PTABLE_BITS = [
    0x3F000000, 0x3EEC515A, 0x3EB9E4E3, 0x3E79375C, 0x3E0E5ACB, 0x3D8A9501,
    0x3CE5ED93, 0x3C2289CB, 0x3B43D285, 0x3A4909DD, 0x392FE09E, 0x38031DFC,
    0x36A696B8, 0x35345CD8, 0x33A6674D, 0x3202D2C5, 0x302F4A31, 0x2E4824C7,
    0x2C42BB52, 0x2A2173E9, 0x27E4229E, 0x258959AD, 0x230CEE5E, 0x207672F6,
    0x1DB79FE2, 0x1AE92B5E,
]


def _prob_table() -> np.ndarray:
    """p(a) for a = 0,2,...,50, bit-matching the reference's jnp.exp."""
    try:
        import jax.numpy as jnp

        a = np.arange(0, 51, 2, dtype=np.float32)
        p = np.asarray(0.5 * jnp.exp(-(jnp.asarray(a) * a) / (2.0 * 5.0**2)),
                       dtype=np.float32)
        if p.shape == (26,) and np.all(np.diff(p) < 0):
            return p
    except Exception:
        pass
    return np.array(_PTABLE_BITS, dtype=np.uint32).view(np.float32)


def _flip_thresholds(u: np.ndarray, ptable: np.ndarray) -> np.ndarray:
    """A(u): flip <=> |h| < A. A = 52 - 2 * #{a : p(a) <= u}."""
    tab = ptable[::-1].copy()  # ascending: p(50), p(48), ..., p(0)
    idx = np.searchsorted(tab, u, side="right")
    return (52 - 2 * idx).astype(np.int32)


# Batch strips per core (software pipeline): small first strip so the PE
# starts quickly behind the DMA transient, small last strip so the drain
# (noisy-sign chains with no L1 work left to hide them) is short.
WIDTHS = [512, 512, 448, 320, 256]
OFFS = [sum(WIDTHS[:i]) for i in range(len(WIDTHS))]
ST = len(WIDTHS)
SWMAX = max(WIDTHS)

# Layer-1 operand: x = three 4-bit base-16 digits of round(x*2^21)
# (12 bits, stored at fp8-e4m3-exact scales d*2^-1 / 2^-5 / 2^-9, rows
# padded 3*784 -> 2560 = 10 DoubleRow chunks at 0.5 cy/row) plus an fp16
# residual (|r| <= 2^-10, exact to ~2^-23, 7 chunks at 1.0) accumulating
# into the SAME psum: 12 chunk-equivalents vs 13 for the Dekker split.
# The residual pass reuses the digit weight tile: its rows 0..783 are
# the plain +-1 signs.
GROWS = 2560              # padded digit rows (3*784 -> 10*256)
NDR = GROWS // 256        # 10 DR chunks
NHC = GROWS // 128        # 20 half-chunks


def emit_ns_front(nc, pool, ps, sw, a_ap, nb, use_hb, cg_bufs=2):
    """First three ops of v = noisy_sign(h) = Sign(h - s*A'): s, m, w.

    ps holds h (exact even integers). s = Sign(ps - 0.5) in {+-1} (maps
    h==0 to -1 like the reference). a_ap holds A' = A-1 (odd, in [-1,51],
    exact in bf16). Returns the w tile; the final Sign is emitted one
    o-round later (emit_ns_back) so the in-order ACT queue never stalls
    waiting for the DVE middle ops.
    """
    s = pool.tile([128, SWMAX], BF16, tag="s", bufs=cg_bufs)
    nc.scalar.activation(s[:, :sw], ps[:], ACTF.Sign, bias=nb)
    m = pool.tile([128, SWMAX], BF16, tag="m", bufs=cg_bufs)
    nc.vector.tensor_tensor(m[:, :sw], s[:, :sw], a_ap, op=ALU.mult)
    w = pool.tile([128, SWMAX], BF16, tag="w", bufs=cg_bufs)
    if use_hb:
        # ACT has slack in the drain: copy h to bf16 so the DVE subtract
        # runs in 2x 16-bit mode. Exact: bf16 holds even ints <= 512
        # exactly, and for |h| > 512 the +-2 rounding cannot flip
        # sign(h - s*A') since |w| >= 461.
        hb = pool.tile([128, SWMAX], BF16, tag="hb", bufs=cg_bufs)
        nc.scalar.activation(hb[:, :sw], ps[:], ACTF.Copy)
        nc.vector.tensor_tensor(w[:, :sw], hb[:, :sw], m[:, :sw],
                                op=ALU.subtract)
    else:
        nc.vector.tensor_tensor(w[:, :sw], ps[:], m[:, :sw],
                                op=ALU.subtract)
    return w


def emit_ns_back(nc, w, sw, out_ap, on_dve=False):
    """Final op of the noisy-sign chain: w = h - s*A' is an odd integer,
    never 0 (bf16 rounding of large |w| cannot cross 0). Either ACT Sign
    or an exact DVE clamp(w, -1, 1) - equal because |w| >= 1."""
    if on_dve:
        nc.vector.tensor_scalar(out_ap, w[:, :sw], 1.0, -1.0,
                                op0=ALU.min, op1=ALU.max)
    else:
        nc.scalar.activation(out_ap, w[:, :sw], ACTF.Sign)


def build_nc(repeat: int = 1):
    """Build the per-core Bass program (same program on all 8 cores)."""
    nc = bacc.Bacc("TRN2", target_bir_lowering=False, debug=False,
                   num_devices=N_CORES)

    xq = nc.dram_tensor("xq", [GROWS, BC], FP8, kind="ExternalInput").ap()
    xr = nc.dram_tensor("xr", [D_IN, BC], F16, kind="ExternalInput").ap()
    a2 = nc.dram_tensor("a2", [D_H, BC], BF16, kind="ExternalInput").ap()
    a3 = nc.dram_tensor("a3", [D_H, BC], BF16, kind="ExternalInput").ap()
    w1 = nc.dram_tensor("w1", [GROWS, D_H], FP8, kind="ExternalInput").ap()
    w2 = nc.dram_tensor("w2", [D_H, D_H], FP8, kind="ExternalInput").ap()
    w3 = nc.dram_tensor("w3", [D_H, D_H], FP8, kind="ExternalInput").ap()
    w4 = nc.dram_tensor("w4", [D_H, 16], FP8, kind="ExternalInput").ap()
    c1 = nc.dram_tensor("c1", [128, OC], F32, kind="ExternalInput").ap()
    out = nc.dram_tensor("out", [D_OUT, BC], F32, kind="ExternalOutput").ap()

    with tile.TileContext(nc) as tc:
        with ExitStack() as ctx:
            consts = ctx.enter_context(tc.tile_pool(name="consts", bufs=1))
            xp = ctx.enter_context(tc.tile_pool(name="xp", bufs=1))
            # Per-strip +-1 panels: written in one period, read the next.
            bp = ctx.enter_context(tc.tile_pool(name="bp", bufs=2))
            l1ps = ctx.enter_context(
                tc.tile_pool(name="l1ps", bufs=3, space="PSUM"))
            l2ps = ctx.enter_context(
                tc.tile_pool(name="l2ps", bufs=2, space="PSUM"))
            l3ps = ctx.enter_context(
                tc.tile_pool(name="l3ps", bufs=2, space="PSUM"))
            l4ps = ctx.enter_context(
                tc.tile_pool(name="l4ps", bufs=1, space="PSUM"))
            tmp2 = ctx.enter_context(tc.tile_pool(name="tmp2", bufs=3))
            tmp3 = ctx.enter_context(tc.tile_pool(name="tmp3", bufs=3))
            l4out = ctx.enter_context(tc.tile_pool(name="l4out", bufs=1))

            w1_t = consts.tile([128, NHC, D_H], FP8, tag="w1")
            c1_t = consts.tile([128, OC], F32, tag="c1")
            nb_t = consts.tile([128, 1], F32, tag="nb")
            nc.gpsimd.memset(nb_t[:], -0.5)
            xq_t = xp.tile([128, NHC, BC], FP8, tag="xq")
            xr_t = xp.tile([128, K1, BC], F16, tag="xr")
            w2_t = consts.tile([128, KH, D_H], FP8, tag="w2")
            w3_t = consts.tile([128, KH, D_H], FP8, tag="w3")
            w4_t = consts.tile([128, KH, 16], FP8, tag="w4")
            a2_t = consts.tile([128, KH, BC], BF16, tag="a2")
            a3_t = consts.tile([128, KH, BC], BF16, tag="a3")

            def x_strip(s):
                """Load digit + residual columns for strip s."""
                c0, c1_ = OFFS[s], OFFS[s] + WIDTHS[s]
                nc.sync.dma_start(
                    xq_t[:, :, c0:c1_],
                    xq[:, c0:c1_].rearrange("(k p) m -> p k m", p=128))
                nc.sync.dma_start(
                    xr_t[:, :K1 - 1, c0:c1_],
                    xr[0:128 * (K1 - 1), c0:c1_].rearrange(
                        "(k p) m -> p k m", p=128))
                nc.sync.dma_start(xr_t[:K1_LAST, K1 - 1, c0:c1_],
                                  xr[128 * (K1 - 1):D_IN, c0:c1_])

            def a_strip(a_t, a, s):
                c0, c1_ = OFFS[s], OFFS[s] + WIDTHS[s]
                nc.sync.dma_start(
                    a_t[:, :, c0:c1_],
                    a[:, c0:c1_].rearrange("(k p) m -> p k m", p=128))

            # DMA order = consumption order. Strip 0: interleaved 4-chunk
            # groups of w1/x so the k-major period-0 matmuls chase arrivals;
            # grouping amortizes the ~900ns per-DMA semaphore latency.
            w0 = WIDTHS[0]
            nc.sync.dma_start(w1_t[:, 0:4],
                              w1[0:512, :].rearrange("(k p) m -> p k m",
                                                     p=128))
            for g in range(5):
                nc.sync.dma_start(
                    xq_t[:, 4 * g:4 * g + 4, 0:w0],
                    xq[512 * g:512 * g + 512, 0:w0].rearrange(
                        "(k p) m -> p k m", p=128))
                if g >= 1:
                    nc.sync.dma_start(
                        w1_t[:, 4 * g:4 * g + 4],
                        w1[512 * g:512 * g + 512, :].rearrange(
                            "(k p) m -> p k m", p=128))
            nc.sync.dma_start(
                xr_t[:, :K1 - 1, 0:w0],
                xr[0:128 * (K1 - 1), 0:w0].rearrange("(k p) m -> p k m",
                                                     p=128))
            nc.sync.dma_start(xr_t[:K1_LAST, K1 - 1, 0:w0],
                              xr[128 * (K1 - 1):D_IN, 0:w0])
            nc.gpsimd.dma_start(c1_t[:], c1[:, :])
            nc.sync.dma_start(w2_t[:, :, :],
                              w2.rearrange("(k p) m -> p k m", p=128))
            x_strip(1)
            a_strip(a2_t, a2, 0)
            nc.sync.dma_start(w3_t[:, :, :],
                              w3.rearrange("(k p) m -> p k m", p=128))
            nc.sync.dma_start(w4_t[:, :, :],
                              w4.rearrange("(k p) m -> p k m", p=128))
            x_strip(2)
            a_strip(a2_t, a2, 1)
            a_strip(a3_t, a3, 0)
            x_strip(3)
            a_strip(a2_t, a2, 2)
            a_strip(a3_t, a3, 1)
            x_strip(4)
            a_strip(a2_t, a2, 3)
            a_strip(a3_t, a3, 2)
            a_strip(a2_t, a2, 4)
            a_strip(a3_t, a3, 3)
            a_strip(a3_t, a3, 4)

            # Software pipeline over batch strips: period p runs L1(p),
            # L2(p-1), L3(p-2), L4(p-3), interleaved per o-round so PE never
            # waits on the ACT/DVE noisy-sign chains.
            sb1, sb2, sb3 = {}, {}, {}
            for p in range(ST + 3):
                s1, s2, s3, s4 = p, p - 1, p - 2, p - 3
                if 0 <= s1 < ST:
                    sb1[s1] = bp.tile([128, KH, SWMAX], FP8, tag="b1",
                                      name="b1")
                if 0 <= s2 < ST:
                    sb2[s2] = bp.tile([128, KH, SWMAX], FP8, tag="b2",
                                      name="b2")
                if 0 <= s3 < ST:
                    sb3[s3] = bp.tile([128, KH, SWMAX], FP8, tag="b3",
                                      name="b3")
                if 0 <= s4 < ST:
                    ps4 = l4ps.tile([16, SWMAX], F32, tag="mm4")

                if p == 0:
                    # Period 0 is DMA-chased: emit k-major over psum pairs
                    # so both psums accumulate each chunk as its DMA lands.
                    w = WIDTHS[0]
                    ssl = slice(OFFS[0], OFFS[0] + w)
                    # Borrow the idle L2/L3 psum banks: six psums chase the
                    # startup DMA k-major (15us of PE work vs ~13us of DMA),
                    # then the last o-pair runs on resident data.
                    for grp in ([(l1ps, "mm1", 0), (l2ps, "mm2", 2),
                                 (l3ps, "mm3", 4)], [(l1ps, "mm1", 6)]):
                        pss = []
                        for pool_, tag_, og in grp:
                            for i in range(2):
                                pss.append((og + i, pool_.tile(
                                    [128, SWMAX], F32, tag=tag_,
                                    name="ps0")))
                        for k in range(NDR + K1):
                            for o, pst in pss:
                                osl0 = slice(o * 128, (o + 1) * 128)
                                if k < NDR:
                                    nc.tensor.matmul(
                                        pst[:, :w],
                                        w1_t[:, 2 * k:2 * k + 2, osl0],
                                        xq_t[:, 2 * k:2 * k + 2, ssl],
                                        start=(k == 0),
                                        stop=False,
                                        perf_mode=DR,
                                    )
                                else:
                                    c = k - NDR
                                    kk = K1_LAST if c == K1 - 1 else 128
                                    nc.tensor.matmul(
                                        pst[:, :w],
                                        w1_t[:kk, c, osl0],
                                        xr_t[:kk, c, ssl],
                                        start=False,
                                        stop=(c == K1 - 1),
                                    )
                        for o, pst in pss:
                            nc.scalar.activation(sb1[0][:, o, :w],
                                                 pst[:, :w], ACTF.Sign,
                                                 bias=c1_t[:, o:o + 1])

                drain_v = (p >= ST - 2) and _os.environ.get(
                    "KDVEV", "1") == "1"
                use_hb = drain_v and _os.environ.get("KHB", "0") == "1"
                pend2 = pend3 = None
                for o in range(OC):
                    osl = slice(o * 128, (o + 1) * 128)
                    if 0 < s1 < ST:
                        w = WIDTHS[s1]
                        ssl = slice(OFFS[s1], OFFS[s1] + w)
                        ps = l1ps.tile([128, SWMAX], F32, tag="mm1")
                        for c in range(NDR):
                            nc.tensor.matmul(
                                ps[:, :w],
                                w1_t[:, 2 * c:2 * c + 2, osl],
                                xq_t[:, 2 * c:2 * c + 2, ssl],
                                start=(c == 0),
                                stop=False,
                                perf_mode=DR,
                            )
                        for c in range(K1):
                            kk = K1_LAST if c == K1 - 1 else 128
                            nc.tensor.matmul(
                                ps[:, :w],
                                w1_t[:kk, c, osl],
                                xr_t[:kk, c, ssl],
                                start=False,
                                stop=(c == K1 - 1),
                            )
                        # b1 = sign(h1 - mu1); c1 arrives negated so ACT
                        # computes Sign(h + (-mu1)) in one op.
                        nc.scalar.activation(sb1[s1][:, o, :w], ps[:, :w],
                                             ACTF.Sign, bias=c1_t[:, o:o + 1])
                    if 0 <= s2 < ST:
                        w = WIDTHS[s2]
                        ps = l2ps.tile([128, SWMAX], F32, tag="mm2")
                        for kp in range(KH // 2):
                            nc.tensor.matmul(
                                ps[:, :w],
                                w2_t[:, 2 * kp:2 * kp + 2, osl],
                                sb1[s2][:, 2 * kp:2 * kp + 2, :w],
                                start=(kp == 0),
                                stop=(kp == KH // 2 - 1),
                                perf_mode=DR,
                            )
                        wt = emit_ns_front(
                            nc, tmp2, ps[:, :w], w,
                            a2_t[:, o, OFFS[s2]:OFFS[s2] + w], nb_t[:],
                            use_hb)
                        dve_v = drain_v and (o % 2 == 1)
                        if pend2 is not None:
                            emit_ns_back(nc, *pend2)
                        pend2 = (wt, w, sb2[s2][:, o, :w], dve_v)
                    if 0 <= s3 < ST:
                        w = WIDTHS[s3]
                        ps = l3ps.tile([128, SWMAX], F32, tag="mm3")
                        for kp in range(KH // 2):
                            nc.tensor.matmul(
                                ps[:, :w],
                                w3_t[:, 2 * kp:2 * kp + 2, osl],
                                sb2[s3][:, 2 * kp:2 * kp + 2, :w],
                                start=(kp == 0),
                                stop=(kp == KH // 2 - 1),
                                perf_mode=DR,
                            )
                        wt = emit_ns_front(
                            nc, tmp3, ps[:, :w], w,
                            a3_t[:, o, OFFS[s3]:OFFS[s3] + w], nb_t[:],
                            use_hb)
                        dve_v = drain_v and (o % 2 == 0)
                        if pend3 is not None:
                            emit_ns_back(nc, *pend3)
                        pend3 = (wt, w, sb3[s3][:, o, :w], dve_v)
                    if 0 <= s4 < ST and o < KH // 2:
                        w = WIDTHS[s4]
                        nc.tensor.matmul(
                            ps4[:, :w],
                            w4_t[:, 2 * o:2 * o + 2, :],
                            sb3[s4][:, 2 * o:2 * o + 2, :w],
                            start=(o == 0),
                            stop=(o == KH // 2 - 1),
                            perf_mode=DR,
                        )
                        if o == KH // 2 - 1:
                            ot = l4out.tile([D_OUT, SWMAX], F32, tag="ot")
                            nc.scalar.activation(ot[:, :w], ps4[:D_OUT, :w],
                                                 ACTF.Copy)
                            nc.sync.dma_start(
                                out[:, OFFS[s4]:OFFS[s4] + w], ot[:, :w])
                if pend2 is not None:
                    emit_ns_back(nc, *pend2)
                if pend3 is not None:
                    emit_ns_back(nc, *pend3)

    nc.compile()
    return nc


_NC_CACHE: dict[int, object] = {}


def _get_nc(repeat: int = 1):
    if repeat not in _NC_CACHE:
        _NC_CACHE[repeat] = build_nc(repeat)
    return _NC_CACHE[repeat]


def make_in_maps(x, u2, u3, W1, W2, W3, W4, **_unused):
    """Host preprocessing -> per-core input dicts."""
    fp8_np = mybir.dt.np(FP8)
    bf16_np = mybir.dt.np(BF16)

    x = np.asarray(x, dtype=np.float32)
    W1b = np.sign(np.asarray(W1, dtype=np.float32))
    # mean(h1, axis=0) = sign(W1) @ mean(x, axis=0), in float64.
    mu1 = (W1b.astype(np.float64) @ x.mean(axis=0, dtype=np.float64)).astype(
        np.float32)
    # negated: the device computes Sign(h + bias) with bias = -mu1
    c1 = np.ascontiguousarray((-mu1).reshape(OC, 128).T)  # [128, OC]

    # Three 4-bit base-16 digits of round(x*2^21) at fp8-exact scales,
    # plus the fp16 residual (|r| <= ~2^-10, exact to ~2^-23).
    q = np.round(x.astype(np.float64) * (1 << 21)).astype(np.int64)
    r = q.T.copy()                                       # [784, B]
    digs = []
    for j in range(3):
        sc = 1 << (4 * (5 - j))
        d = np.floor_divide(2 * r + sc, 2 * sc)          # round-half-up
        r -= d * sc
        digs.append(d)
    assert all(np.abs(d).max() <= 15 for d in digs)
    xqt = np.zeros((GROWS, x.shape[0]), dtype=fp8_np)    # [2560, B]
    for i in range(3):
        v = digs[i].astype(np.float32) * 2.0**(-1 - 4 * i)
        xqt[D_IN * i:D_IN * (i + 1)] = v.astype(fp8_np)
    dig_sum = sum(d.astype(np.float64) * 2.0**(-1 - 4 * i)
                  for i, d in enumerate(digs))
    xrt = (x.T.astype(np.float64) - dig_sum).astype(np.float16)

    pt = _prob_table()
    a2i = _flip_thresholds(np.asarray(u2), pt) - 1   # A' = A-1, odd
    a3i = _flip_thresholds(np.asarray(u3), pt) - 1

    a2t = np.ascontiguousarray(a2i.T.astype(bf16_np))    # [1024, B] bf16
    a3t = np.ascontiguousarray(a3i.T.astype(bf16_np))
    w1b8 = W1b.T.astype(fp8_np)
    w1t = np.zeros((GROWS, D_H), dtype=fp8_np)           # [2560, 1024] fp8
    for i in range(3):
        w1t[D_IN * i:D_IN * (i + 1)] = w1b8
    w2t = np.ascontiguousarray(
        np.sign(np.asarray(W2, np.float32)).T).astype(fp8_np)
    w3t = np.ascontiguousarray(
        np.sign(np.asarray(W3, np.float32)).T).astype(fp8_np)
    w4t = np.zeros((D_H, 16), dtype=fp8_np)
    w4t[:, :D_OUT] = np.ascontiguousarray(
        np.sign(np.asarray(W4, np.float32)).T).astype(fp8_np)

    in_maps = []
    for c in range(N_CORES):
        sl = slice(c * BC, (c + 1) * BC)
        in_maps.append({
            "xq": np.ascontiguousarray(xqt[:, sl]),
            "xr": np.ascontiguousarray(xrt[:, sl]),
            "a2": np.ascontiguousarray(a2t[:, sl]),
            "a3": np.ascontiguousarray(a3t[:, sl]),
            "w1": w1t,
            "w2": w2t,
            "w3": w3t,
            "w4": w4t,
            "c1": c1,
        })
    return in_maps


def kernel(x, u2, u3, W1, W2, W3, W4,
           g1=None, b1=None, g2=None, b2=None, g3=None, b3=None):
    for g in (g1, g2, g3):
        assert g is None or np.all(np.asarray(g) > 0), "kernel assumes g > 0"
    for b in (b1, b2, b3):
        assert b is None or np.all(np.asarray(b) == 0), "kernel assumes b == 0"

    nc = _get_nc(repeat=1)
    in_maps = make_in_maps(x, u2, u3, W1, W2, W3, W4)
    res = run_bass_kernel_spmd(nc, in_maps, core_ids=list(range(N_CORES)))

    out = np.empty((B, D_OUT), dtype=np.float32)
    for c in range(N_CORES):
        out[c * BC:(c + 1) * BC, :] = res.results[c]["out"].T
    return out


# revision 82
# speedup vs baseline: 1.3490x; 1.0051x over previous
"""Bass/Trainium2 kernel for a binarized NN (BNN) forward pass, data-parallel
over 8 NeuronCores.

Reference semantics (fp32):
    h1 = x @ sign(W1).T;  b1 = sign(h1 - mean(h1, axis=0))        # g=1, b=0
    h2 = b1 @ sign(W2).T; b2 = noisy_sign(h2, u2)                  # BN+sign is
    h3 = b2 @ sign(W3).T; b3 = noisy_sign(h3, u3)                  # identity on +-1
    out = b3 @ sign(W4).T

Math facts exploited:
  * b in {+-1,0} and sign(W) in {+-1} make h2/h3/out exact small integers under
    fp32 PSUM accumulation in any order -> fp8 (e4m3) matmuls on PE are
    bit-exact, enabling DoubleRow perf mode.
  * batchnorm+sign on +-1 inputs is the identity (|mean| < 1), so no batch
    statistics and no cross-core communication are needed for layers 2/3.
  * mean(h1, axis=0) == mean(x, axis=0) @ sign(W1).T -> computed on host in
    float64 (tiny dot), passed in as a per-feature threshold c1.
  * Layer 1 runs as TWO fp16 matmul passes on an exact Dekker split
    x = x_hi + x_lo (12+12 mantissa bits; fp16 operands, incl. subnormals,
    are honored exactly by the PE path - probed). This reproduces the fp32
    h1 to ~1e-7, vs the reference's own ~2e-6 chunked-PSUM rounding noise;
    measured ~0-2 borderline sign differences across the full batch, each
    perturbing one batch row (~0.006 rel err) - far inside the 2e-2 gate.
    fp16 matmuls cost 1 PE cycle/row vs fp32's 4.
  * The stochastic flip (u < 0.5*exp(-h^2/50)) & (|h| <= 50) with h an exact
    even integer depends only on |h| in {0,2,...,50}: precompute on host
    A(u) = smallest even a with p(a) <= u, then flip <=> |h| < A. With
    A' = A-1 (odd) and s = sign-with-0-to-minus(h) = Sign(h - 0.5):
        noisy_sign(h) = Sign(h - s*A')        (h - s*A' is odd, never 0)
    so each chain is ACT Sign -> DVE mult -> DVE subtract -> ACT Sign and
    panels hold +-1 directly (no scaling factors anywhere).

Per-core layout is feature-major ("transposed"): activations live as
[features(partitions), batch(free)], so batch stays on the free dim and no
on-device transposes are needed. Batch 16384 is sharded 2048/core.
"""

from contextlib import ExitStack

import numpy as np

import concourse.bass as bass  # noqa: F401
import concourse.tile as tile
from concourse import bacc, mybir
from concourse.bass_utils import run_bass_kernel_spmd

F32 = mybir.dt.float32
F16 = mybir.dt.float16
BF16 = mybir.dt.bfloat16
FP8 = mybir.dt.float8e4
ALU = mybir.AluOpType
ACTF = mybir.ActivationFunctionType
DR = mybir.MatmulPerfMode.DoubleRow

N_CORES = 8
B = 16384                 # full batch
BC = B // N_CORES         # batch per core
D_IN = 784                # layer-1 input features
D_H = 1024                # hidden features
D_OUT = 10                # output features
K1 = (D_IN + 127) // 128  # 7 k-chunks for layer 1 (6 full + 16 rows)
K1_LAST = D_IN - 128 * (K1 - 1)
KH = D_H // 128           # 8 k-chunks for hidden layers
OC = D_H // 128           # 8 output-feature chunks
HB = BC // 2              # half-batch chain width (1024)

# float32(0.5*exp(-(a*a)/50)) for a = 0,2,...,50, computed with jnp.exp on the
# same jax backend the reference uses (fallback if jax is unavailable here).
_PTABLE_BITS = [
    0x3F000000, 0x3EEC515A, 0x3EB9E4E3, 0x3E79375C, 0x3E0E5ACB, 0x3D8A9501,
    0x3CE5ED93, 0x3C2289CB, 0x3B43D285, 0x3A4909DD, 0x392FE09E, 0x38031DFC,
    0x36A696B8, 0x35345CD8, 0x33A6674D, 0x3202D2C5, 0x302F4A31, 0x2E4824C7,
    0x2C42BB52, 0x2A2173E9, 0x27E4229E, 0x258959AD, 0x230CEE5E, 0x207672F6,
    0x1DB79FE2, 0x1AE92B5E,
]


def _prob_table() -> np.ndarray:
    """p(a) for a = 0,2,...,50, bit-matching the reference's jnp.exp."""
    try:
        import jax.numpy as jnp

        a = np.arange(0, 51, 2, dtype=np.float32)
        p = np.asarray(0.5 * jnp.exp(-(jnp.asarray(a) * a) / (2.0 * 5.0**2)),
                       dtype=np.float32)
        if p.shape == (26,) and np.all(np.diff(p) < 0):
            return p
    except Exception:
        pass
    return np.array(_PTABLE_BITS, dtype=np.uint32).view(np.float32)


def _flip_thresholds(u: np.ndarray, ptable: np.ndarray) -> np.ndarray:
    """A(u): flip <=> |h| < A. A = 52 - 2 * #{a : p(a) <= u}."""
    tab = ptable[::-1].copy()  # ascending: p(50), p(48), ..., p(0)
    idx = np.searchsorted(tab, u, side="right")
    return (52 - 2 * idx).astype(np.int32)


# Batch strips per core (software pipeline): small first strip so the PE
# starts quickly behind the DMA transient, small last strip so the drain
# (noisy-sign chains with no L1 work left to hide them) is short.
WIDTHS = [512, 512, 448, 320, 256]
OFFS = [sum(WIDTHS[:i]) for i in range(len(WIDTHS))]
ST = len(WIDTHS)
SWMAX = max(WIDTHS)

# Layer-1 operand: x = three 4-bit base-16 digits of round(x*2^21)
# (12 bits, stored at fp8-e4m3-exact scales d*2^-1 / 2^-5 / 2^-9, rows
# padded 3*784 -> 2560 = 10 DoubleRow chunks at 0.5 cy/row) plus an fp16
# residual (|r| <= 2^-10, exact to ~2^-23, 7 chunks at 1.0) accumulating
# into the SAME psum: 12 chunk-equivalents vs 13 for the Dekker split.
# The residual pass reuses the digit weight tile: its rows 0..783 are
# the plain +-1 signs.
GROWS = 2560              # padded digit rows (3*784 -> 10*256)
NDR = GROWS // 256        # 10 DR chunks
NHC = GROWS // 128        # 20 half-chunks


def emit_ns_front(nc, pool, ps, sw, a_ap, nb, use_hb, cg_bufs=2):
    """First three ops of v = noisy_sign(h) = Sign(h - s*A'): s, m, w.

    ps holds h (exact even integers). s = Sign(ps - 0.5) in {+-1} (maps
    h==0 to -1 like the reference). a_ap holds A' = A-1 (odd, in [-1,51],
    exact in bf16). Returns the w tile; the final Sign is emitted one
    o-round later (emit_ns_back) so the in-order ACT queue never stalls
    waiting for the DVE middle ops.
    """
    s = pool.tile([128, SWMAX], BF16, tag="s", bufs=cg_bufs)
    nc.scalar.activation(s[:, :sw], ps[:], ACTF.Sign, bias=nb)
    m = pool.tile([128, SWMAX], BF16, tag="m", bufs=cg_bufs)
    nc.vector.tensor_tensor(m[:, :sw], s[:, :sw], a_ap, op=ALU.mult)
    w = pool.tile([128, SWMAX], BF16, tag="w", bufs=cg_bufs)
    if use_hb:
        # ACT has slack in the drain: copy h to bf16 so the DVE subtract
        # runs in 2x 16-bit mode. Exact: bf16 holds even ints <= 512
        # exactly, and for |h| > 512 the +-2 rounding cannot flip
        # sign(h - s*A') since |w| >= 461.
        hb = pool.tile([128, SWMAX], BF16, tag="hb", bufs=cg_bufs)
        nc.scalar.activation(hb[:, :sw], ps[:], ACTF.Copy)
        nc.vector.tensor_tensor(w[:, :sw], hb[:, :sw], m[:, :sw],
                                op=ALU.subtract)
    else:
        nc.vector.tensor_tensor(w[:, :sw], ps[:], m[:, :sw],
                                op=ALU.subtract)
    return w


def emit_ns_back(nc, w, sw, out_ap, on_dve=False):
    """Final op of the noisy-sign chain: w = h - s*A' is an odd integer,
    never 0 (bf16 rounding of large |w| cannot cross 0). Either ACT Sign
    or an exact DVE clamp(w, -1, 1) - equal because |w| >= 1."""
    if on_dve:
        nc.vector.tensor_scalar(out_ap, w[:, :sw], 1.0, -1.0,
                                op0=ALU.min, op1=ALU.max)
    else:
        nc.scalar.activation(out_ap, w[:, :sw], ACTF.Sign)


def build_nc(repeat: int = 1):
    """Build the per-core Bass program (same program on all 8 cores)."""
    nc = bacc.Bacc("TRN2", target_bir_lowering=False, debug=False,
                   num_devices=N_CORES)

    xq = nc.dram_tensor("xq", [GROWS, BC], FP8, kind="ExternalInput").ap()
    xr = nc.dram_tensor("xr", [D_IN, BC], F16, kind="ExternalInput").ap()
    a2 = nc.dram_tensor("a2", [D_H, BC], BF16, kind="ExternalInput").ap()
    a3 = nc.dram_tensor("a3", [D_H, BC], BF16, kind="ExternalInput").ap()
    w1 = nc.dram_tensor("w1", [GROWS, D_H], FP8, kind="ExternalInput").ap()
    w2 = nc.dram_tensor("w2", [D_H, D_H], FP8, kind="ExternalInput").ap()
    w3 = nc.dram_tensor("w3", [D_H, D_H], FP8, kind="ExternalInput").ap()
    w4 = nc.dram_tensor("w4", [D_H, 16], FP8, kind="ExternalInput").ap()
    c1 = nc.dram_tensor("c1", [128, OC], F32, kind="ExternalInput").ap()
    out = nc.dram_tensor("out", [D_OUT, BC], F32, kind="ExternalOutput").ap()

    with tile.TileContext(nc) as tc:
        with ExitStack() as ctx:
            consts = ctx.enter_context(tc.tile_pool(name="consts", bufs=1))
            xp = ctx.enter_context(tc.tile_pool(name="xp", bufs=1))
            # Per-strip +-1 panels: written in one period, read the next.
            bp = ctx.enter_context(tc.tile_pool(name="bp", bufs=2))
            l1ps = ctx.enter_context(
                tc.tile_pool(name="l1ps", bufs=2, space="PSUM"))
            l2ps = ctx.enter_context(
                tc.tile_pool(name="l2ps", bufs=2, space="PSUM"))
            l3ps = ctx.enter_context(
                tc.tile_pool(name="l3ps", bufs=3, space="PSUM"))
            l4ps = ctx.enter_context(
                tc.tile_pool(name="l4ps", bufs=1, space="PSUM"))
            tmp2 = ctx.enter_context(tc.tile_pool(name="tmp2", bufs=3))
            tmp3 = ctx.enter_context(tc.tile_pool(name="tmp3", bufs=3))
            l4out = ctx.enter_context(tc.tile_pool(name="l4out", bufs=1))

            w1_t = consts.tile([128, NHC, D_H], FP8, tag="w1")
            c1_t = consts.tile([128, OC], F32, tag="c1")
            nb_t = consts.tile([128, 1], F32, tag="nb")
            nc.gpsimd.memset(nb_t[:], -0.5)
            xq_t = xp.tile([128, NHC, BC], FP8, tag="xq")
            xr_t = xp.tile([128, K1, BC], F16, tag="xr")
            w2_t = consts.tile([128, KH, D_H], FP8, tag="w2")
            w3_t = consts.tile([128, KH, D_H], FP8, tag="w3")
            w4_t = consts.tile([128, KH, 16], FP8, tag="w4")
            a2_t = consts.tile([128, KH, BC], BF16, tag="a2")
            a3_t = consts.tile([128, KH, BC], BF16, tag="a3")

            def x_strip(s):
                """Load digit + residual columns for strip s."""
                c0, c1_ = OFFS[s], OFFS[s] + WIDTHS[s]
                nc.sync.dma_start(
                    xq_t[:, :, c0:c1_],
                    xq[:, c0:c1_].rearrange("(k p) m -> p k m", p=128))
                nc.sync.dma_start(
                    xr_t[:, :K1 - 1, c0:c1_],
                    xr[0:128 * (K1 - 1), c0:c1_].rearrange(
                        "(k p) m -> p k m", p=128))
                nc.sync.dma_start(xr_t[:K1_LAST, K1 - 1, c0:c1_],
                                  xr[128 * (K1 - 1):D_IN, c0:c1_])

            def a_strip(a_t, a, s):
                c0, c1_ = OFFS[s], OFFS[s] + WIDTHS[s]
                nc.sync.dma_start(
                    a_t[:, :, c0:c1_],
                    a[:, c0:c1_].rearrange("(k p) m -> p k m", p=128))

            # DMA order = consumption order. Strip 0: interleaved 4-chunk
            # groups of w1/x so the k-major period-0 matmuls chase arrivals;
            # grouping amortizes the ~900ns per-DMA semaphore latency.
            w0 = WIDTHS[0]
            nc.sync.dma_start(w1_t[:, 0:4],
                              w1[0:512, :].rearrange("(k p) m -> p k m",
                                                     p=128))
            for g in range(5):
                nc.sync.dma_start(
                    xq_t[:, 4 * g:4 * g + 4, 0:w0],
                    xq[512 * g:512 * g + 512, 0:w0].rearrange(
                        "(k p) m -> p k m", p=128))
                if g >= 1:
                    nc.sync.dma_start(
                        w1_t[:, 4 * g:4 * g + 4],
                        w1[512 * g:512 * g + 512, :].rearrange(
                            "(k p) m -> p k m", p=128))
            nc.sync.dma_start(
                xr_t[:, :K1 - 1, 0:w0],
                xr[0:128 * (K1 - 1), 0:w0].rearrange("(k p) m -> p k m",
                                                     p=128))
            nc.sync.dma_start(xr_t[:K1_LAST, K1 - 1, 0:w0],
                              xr[128 * (K1 - 1):D_IN, 0:w0])
            nc.gpsimd.dma_start(c1_t[:], c1[:, :])
            nc.sync.dma_start(w2_t[:, :, :],
                              w2.rearrange("(k p) m -> p k m", p=128))
            x_strip(1)
            a_strip(a2_t, a2, 0)
            nc.sync.dma_start(w3_t[:, :, :],
                              w3.rearrange("(k p) m -> p k m", p=128))
            nc.sync.dma_start(w4_t[:, :, :],
                              w4.rearrange("(k p) m -> p k m", p=128))
            x_strip(2)
            a_strip(a2_t, a2, 1)
            a_strip(a3_t, a3, 0)
            x_strip(3)
            a_strip(a2_t, a2, 2)
            a_strip(a3_t, a3, 1)
            x_strip(4)
            a_strip(a2_t, a2, 3)
            a_strip(a3_t, a3, 2)
            a_strip(a2_t, a2, 4)
            a_strip(a3_t, a3, 3)
            a_strip(a3_t, a3, 4)

            # Software pipeline over batch strips: period p runs L1(p),
            # L2(p-1), L3(p-2), L4(p-3), interleaved per o-round so PE never
            # waits on the ACT/DVE noisy-sign chains.
            sb1, sb2, sb3 = {}, {}, {}
            for p in range(ST + 3):
                s1, s2, s3, s4 = p, p - 1, p - 2, p - 3
                if 0 <= s1 < ST:
                    sb1[s1] = bp.tile([128, KH, SWMAX], FP8, tag="b1",
                                      name="b1")
                if 0 <= s2 < ST:
                    sb2[s2] = bp.tile([128, KH, SWMAX], FP8, tag="b2",
                                      name="b2")
                if 0 <= s3 < ST:
                    sb3[s3] = bp.tile([128, KH, SWMAX], FP8, tag="b3",
                                      name="b3")
                if 0 <= s4 < ST:
                    ps4 = l4ps.tile([16, SWMAX], F32, tag="mm4")

                if p == 0:
                    # Period 0 is DMA-chased: emit k-major over psum pairs
                    # so both psums accumulate each chunk as its DMA lands.
                    w = WIDTHS[0]
                    ssl = slice(OFFS[0], OFFS[0] + w)
                    # Borrow the idle L2/L3 psum banks: six psums chase the
                    # startup DMA k-major (15us of PE work vs ~13us of DMA),
                    # then the last o-pair runs on resident data.
                    for grp in ([(l1ps, "mm1", 0), (l2ps, "mm2", 2),
                                 (l3ps, "mm3", 4)], [(l1ps, "mm1", 6)]):
                        pss = []
                        for pool_, tag_, og in grp:
                            for i in range(2):
                                pss.append((og + i, pool_.tile(
                                    [128, SWMAX], F32, tag=tag_,
                                    name="ps0")))
                        for k in range(NDR + K1):
                            for o, pst in pss:
                                osl0 = slice(o * 128, (o + 1) * 128)
                                if k < NDR:
                                    nc.tensor.matmul(
                                        pst[:, :w],
                                        w1_t[:, 2 * k:2 * k + 2, osl0],
                                        xq_t[:, 2 * k:2 * k + 2, ssl],
                                        start=(k == 0),
                                        stop=False,
                                        perf_mode=DR,
                                    )
                                else:
                                    c = k - NDR
                                    kk = K1_LAST if c == K1 - 1 else 128
                                    nc.tensor.matmul(
                                        pst[:, :w],
                                        w1_t[:kk, c, osl0],
                                        xr_t[:kk, c, ssl],
                                        start=False,
                                        stop=(c == K1 - 1),
                                    )
                        for o, pst in pss:
                            nc.scalar.activation(sb1[0][:, o, :w],
                                                 pst[:, :w], ACTF.Sign,
                                                 bias=c1_t[:, o:o + 1])

                drain_v = (p >= ST - 2) and _os.environ.get(
                    "KDVEV", "1") == "1"
                use_hb = drain_v and _os.environ.get("KHB", "0") == "1"
                pend2 = pend3 = None
                for o in range(OC):
                    osl = slice(o * 128, (o + 1) * 128)
                    if 0 < s1 < ST:
                        w = WIDTHS[s1]
                        ssl = slice(OFFS[s1], OFFS[s1] + w)
                        ps = l1ps.tile([128, SWMAX], F32, tag="mm1")
                        for c in range(NDR):
                            nc.tensor.matmul(
                                ps[:, :w],
                                w1_t[:, 2 * c:2 * c + 2, osl],
                                xq_t[:, 2 * c:2 * c + 2, ssl],
                                start=(c == 0),
                                stop=False,
                                perf_mode=DR,
                            )
                        for c in range(K1):
                            kk = K1_LAST if c == K1 - 1 else 128
                            nc.tensor.matmul(
                                ps[:, :w],
                                w1_t[:kk, c, osl],
                                xr_t[:kk, c, ssl],
                                start=False,
                                stop=(c == K1 - 1),
                            )
                        # b1 = sign(h1 - mu1); c1 arrives negated so ACT
                        # computes Sign(h + (-mu1)) in one op.
                        nc.scalar.activation(sb1[s1][:, o, :w], ps[:, :w],
                                             ACTF.Sign, bias=c1_t[:, o:o + 1])
                    if 0 <= s2 < ST:
                        w = WIDTHS[s2]
                        ps = l2ps.tile([128, SWMAX], F32, tag="mm2")
                        for kp in range(KH // 2):
                            nc.tensor.matmul(
                                ps[:, :w],
                                w2_t[:, 2 * kp:2 * kp + 2, osl],
                                sb1[s2][:, 2 * kp:2 * kp + 2, :w],
                                start=(kp == 0),
                                stop=(kp == KH // 2 - 1),
                                perf_mode=DR,
                            )
                        wt = emit_ns_front(
                            nc, tmp2, ps[:, :w], w,
                            a2_t[:, o, OFFS[s2]:OFFS[s2] + w], nb_t[:],
                            use_hb)
                        dve_v = drain_v and (o % 2 == 1)
                        if pend2 is not None:
                            emit_ns_back(nc, *pend2)
                        pend2 = (wt, w, sb2[s2][:, o, :w], dve_v)
                    if 0 <= s3 < ST:
                        w = WIDTHS[s3]
                        ps = l3ps.tile([128, SWMAX], F32, tag="mm3")
                        for kp in range(KH // 2):
                            nc.tensor.matmul(
                                ps[:, :w],
                                w3_t[:, 2 * kp:2 * kp + 2, osl],
                                sb2[s3][:, 2 * kp:2 * kp + 2, :w],
                                start=(kp == 0),
                                stop=(kp == KH // 2 - 1),
                                perf_mode=DR,
                            )
                        wt = emit_ns_front(
                            nc, tmp3, ps[:, :w], w,
                            a3_t[:, o, OFFS[s3]:OFFS[s3] + w], nb_t[:],
                            use_hb)
                        dve_v = drain_v and (o % 2 == 0)
                        if pend3 is not None:
                            emit_ns_back(nc, *pend3)
                        pend3 = (wt, w, sb3[s3][:, o, :w], dve_v)
                    if 0 <= s4 < ST and o < KH // 2:
                        w = WIDTHS[s4]
                        nc.tensor.matmul(
                            ps4[:, :w],
                            w4_t[:, 2 * o:2 * o + 2, :],
                            sb3[s4][:, 2 * o:2 * o + 2, :w],
                            start=(o == 0),
                            stop=(o == KH // 2 - 1),
                            perf_mode=DR,
                        )
                        if o == KH // 2 - 1:
                            ot = l4out.tile([D_OUT, SWMAX], F32, tag="ot")
                            nc.scalar.activation(ot[:, :w], ps4[:D_OUT, :w],
                                                 ACTF.Copy)
                            nc.sync.dma_start(
                                out[:, OFFS[s4]:OFFS[s4] + w], ot[:, :w])
                if pend2 is not None:
                    emit_ns_back(nc, *pend2)
                if pend3 is not None:
                    emit_ns_back(nc, *pend3)

    nc.compile()
    return nc


_NC_CACHE: dict[int, object] = {}


def _get_nc(repeat: int = 1):
    if repeat not in _NC_CACHE:
        _NC_CACHE[repeat] = build_nc(repeat)
    return _NC_CACHE[repeat]


def make_in_maps(x, u2, u3, W1, W2, W3, W4, **_unused):
    """Host preprocessing -> per-core input dicts."""
    fp8_np = mybir.dt.np(FP8)
    bf16_np = mybir.dt.np(BF16)

    x = np.asarray(x, dtype=np.float32)
    W1b = np.sign(np.asarray(W1, dtype=np.float32))
    # mean(h1, axis=0) = sign(W1) @ mean(x, axis=0), in float64.
    mu1 = (W1b.astype(np.float64) @ x.mean(axis=0, dtype=np.float64)).astype(
        np.float32)
    # negated: the device computes Sign(h + bias) with bias = -mu1
    c1 = np.ascontiguousarray((-mu1).reshape(OC, 128).T)  # [128, OC]

    # Three 4-bit base-16 digits of round(x*2^21) at fp8-exact scales,
    # plus the fp16 residual (|r| <= ~2^-10, exact to ~2^-23).
    q = np.round(x.astype(np.float64) * (1 << 21)).astype(np.int64)
    r = q.T.copy()                                       # [784, B]
    digs = []
    for j in range(3):
        sc = 1 << (4 * (5 - j))
        d = np.floor_divide(2 * r + sc, 2 * sc)          # round-half-up
        r -= d * sc
        digs.append(d)
    assert all(np.abs(d).max() <= 15 for d in digs)
    xqt = np.zeros((GROWS, x.shape[0]), dtype=fp8_np)    # [2560, B]
    for i in range(3):
        v = digs[i].astype(np.float32) * 2.0**(-1 - 4 * i)
        xqt[D_IN * i:D_IN * (i + 1)] = v.astype(fp8_np)
    dig_sum = sum(d.astype(np.float64) * 2.0**(-1 - 4 * i)
                  for i, d in enumerate(digs))
    xrt = (x.T.astype(np.float64) - dig_sum).astype(np.float16)

    pt = _prob_table()
    a2i = _flip_thresholds(np.asarray(u2), pt) - 1   # A' = A-1, odd
    a3i = _flip_thresholds(np.asarray(u3), pt) - 1

    a2t = np.ascontiguousarray(a2i.T.astype(bf16_np))    # [1024, B] bf16
    a3t = np.ascontiguousarray(a3i.T.astype(bf16_np))
    w1b8 = W1b.T.astype(fp8_np)
    w1t = np.zeros((GROWS, D_H), dtype=fp8_np)           # [2560, 1024] fp8
    for i in range(3):
        w1t[D_IN * i:D_IN * (i + 1)] = w1b8
    w2t = np.ascontiguousarray(
        np.sign(np.asarray(W2, np.float32)).T).astype(fp8_np)
    w3t = np.ascontiguousarray(
        np.sign(np.asarray(W3, np.float32)).T).astype(fp8_np)
    w4t = np.zeros((D_H, 16), dtype=fp8_np)
    w4t[:, :D_OUT] = np.ascontiguousarray(
        np.sign(np.asarray(W4, np.float32)).T).astype(fp8_np)

    in_maps = []
    for c in range(N_CORES):
        sl = slice(c * BC, (c + 1) * BC)
        in_maps.append({
            "xq": np.ascontiguousarray(xqt[:, sl]),
            "xr": np.ascontiguousarray(xrt[:, sl]),
            "a2": np.ascontiguousarray(a2t[:, sl]),
            "a3": np.ascontiguousarray(a3t[:, sl]),
            "w1": w1t,
            "w2": w2t,
            "w3": w3t,
            "w4": w4t,
            "c1": c1,
        })
    return in_maps


def kernel(x, u2, u3, W1, W2, W3, W4,
           g1=None, b1=None, g2=None, b2=None, g3=None, b3=None):
    for g in (g1, g2, g3):
        assert g is None or np.all(np.asarray(g) > 0), "kernel assumes g > 0"
    for b in (b1, b2, b3):
        assert b is None or np.all(np.asarray(b) == 0), "kernel assumes b == 0"

    nc = _get_nc(repeat=1)
    in_maps = make_in_maps(x, u2, u3, W1, W2, W3, W4)
    res = run_bass_kernel_spmd(nc, in_maps, core_ids=list(range(N_CORES)))

    out = np.empty((B, D_OUT), dtype=np.float32)
    for c in range(N_CORES):
        out[c * BC:(c + 1) * BC, :] = res.results[c]["out"].T
    return out
